# revision 28
# baseline (speedup 1.0000x reference)
"""Trainium2 Bass kernel for nn_MultiHeadAttentionQuantum.

Math simplification (verified vs reference to ~5e-7):
  The per-token quantum feature map RX(x+theta) -> CNOT ring -> <Z_w>
  collapses to products of cosines. With u_w = cos(x_w + theta_w):
      q_0 = u1*u2*...*u7
      q_w = u0*u1*...*uw   (w = 1..7)
  Then per batch: scores = q @ q.T / sqrt(2); attn = softmax(scores);
  out = attn @ q; out' = swapaxes(out,1,2).reshape(S,8);  y = out' @ Wc.T + b.
  Softmax max-subtraction is skipped (|scores| <= 5.7, exp <= 287, safe in
  fp32). Row sums come free as a ones-column in the second matmul.

Sharding: data-parallel over batch: 16 batches -> 8 cores x 2 batches.

The v0 kernel was paced by the ACT engine: softmax needs exp of all 16.7M
scores per batch and ACT runs 1 elem/lane/cycle at 1.2 GHz (~218us/core for
33.5M exps).  This version splits each 3-chunk score group between two
engines: ACT takes 2 chunks (exact spline exp), DVE takes 1 chunk via a
bitcast-exp trick that computes the fp16 BIT PATTERN of exp(z/sqrt2)
directly in one instruction:
  bits16 = int16(z * (2^10*log2e/sqrt2) + (15*2^10 + bias))
(Schraudolph in the fp16 bit domain; max rel err ~3%, which softmax
normalization averages down to <5e-3 end-to-end because attention here is
near-uniform, eff. N ~ 2000-4000 of 4096).  The int16 result is written into
a bitcast view of the fp16 exD tile, so the accum matmul consumes it as fp16
with zero extra passes.  The Pool engine (which cannot read PSUM) takes the
SBUF-side phase-Q vector work off the DVE.

Per-core device pipeline:
  phase Q (per batch): DMA x p-major (token s = 32p + t), add theta+pi/2 per
    wire (DVE per-partition scalar), range-reduce mod 2pi, u = ACT Sin,
    13 strided DVE muls -> q9 [128, T, 9] fp32 (col 8 = ones) + fp16 copy,
    PE-transpose chunks -> qT [128, S] fp16 with the 8 feature rows
    replicated at partition strips 0/32/64/96 (for row-group packing).
  phase A (per batch, per 512-token i-block):
    scores: 3 row-group-packed K=8 fp16 matmuls per group -> PSUM [128,1536]
    exp:    3-engine split as above, PSUM->SBUF fp16
    accum:  col-group-packed matmuls X[32s:32s+9] += q9_j^T @ exp
            (strip s = chunk%4; strips summed later by the sel matmul;
            start=(tj<4) clears each strip's first accumulation, so no
            X memset is needed)
    normalize (software-pipelined one i-block behind): DVE copy X->SBUF,
    4 matmuls vs sel[128,9] (sums the 4 strips AND transposes to
    token-major), DVE reciprocal of the ones-row, DVE scale -> osb.
  phase C (per batch, deferred into the next batch's stream so it overlaps
    that batch's attention): the reference's swapaxes+reshape+combine is
    y[128m+p, j] = sum_e oscr[8*(128*mt+p)+e, k] * Wc[j,e] + b[j] with
    m = (S/1024)k + mt.  All fp16 (single-pass PE): per-mt strided gather
    DMAs into glh (row 8 = ones for the bias), 32 mt-major matmuls vs
    wcb=[Wc.T; b] into one PSUM bank (serial drains — concurrent packed
    drains into ONE bank are fatal on HW), one DVE copy, one strided store.
"""

import numpy as np

import concourse.bass as bass
import concourse.bacc as bacc
import concourse.tile as tile
from concourse import mybir
from concourse.masks import make_identity
from concourse._compat import with_exitstack

F32 = mybir.dt.float32
F16 = mybir.dt.float16
I16 = mybir.dt.int16
AF = mybir.ActivationFunctionType
P = 128
E = 8
E9 = 9
IB = 512          # i-block width (tokens per output accumulation block)
JG = 3            # j-chunks per exp group (3 PSUM banks per scores buffer)
INV_SQRT2 = 0.7071067811865476

# exp split at chunk granularity: per 3-chunk group, the first 2 chunks go
# to ACT (exact spline exp), the last to the DVE bitcast trick.  The scores
# land in SEPARATE PSUM tiles per consumer engine (scA/scD): the tile
# dependency tracker serializes multiple reader engines of one tile (it only
# records the last reader per region), which would chain DVE-exp after
# ACT-exp and pace the whole kernel.  The Pool engine cannot read PSUM, so
# it takes the SBUF-side vector work instead.

# fp16-bit-domain Schraudolph constants: bits = int16(z*EXP_C1 + EXP_C2)
EXP_C1 = float(2.0**10 * np.log2(np.e) * INV_SQRT2)
EXP_C2 = float(15 * 2.0**10 + 0.5 - 45.0)


@with_exitstack
def _body(ctx, tc, x_in, thp, wcb, sel, y, oscr, S, NB):
    nc = tc.nc
    T = S // P                 # token-chunks (tokens per partition)
    NIB = S // IB              # i-blocks per batch
    M4 = S // (P * E)          # row-tiles per combine feature block
    CPI = IB // P              # chunks per i-block (4)

    const = ctx.enter_context(tc.tile_pool(name="const", bufs=1))
    qpool = ctx.enter_context(tc.tile_pool(name="qdata", bufs=1))
    work = ctx.enter_context(tc.tile_pool(name="work", bufs=2))
    expp = ctx.enter_context(tc.tile_pool(name="expp", bufs=4))
    scpA = ctx.enter_context(tc.tile_pool(name="scpA", bufs=2, space="PSUM"))
    scpD = ctx.enter_context(tc.tile_pool(name="scpD", bufs=2, space="PSUM"))
    outps = ctx.enter_context(tc.tile_pool(name="outps", bufs=2, space="PSUM"))

    ident = const.tile([P, P], F32)
    make_identity(nc, ident[:])
    thp_sb = const.tile([P, E], F32)
    nc.sync.dma_start(thp_sb[:], thp[:])
    wcb_sb = const.tile([P, E], F16)
    nc.sync.dma_start(wcb_sb[:], wcb[:])
    sel_sb = const.tile([P, E9], F16)
    nc.sync.dma_start(sel_sb[:], sel[:])

    q9 = [qpool.tile([P, T * E9], F32, name=f"q9_{b}") for b in range(NB)]
    q9h = [qpool.tile([P, T * E9], F16, name=f"q9h_{b}") for b in range(NB)]
    qT = [qpool.tile([P, S], F16, name=f"qT_{b}") for b in range(NB)]
    osb = [qpool.tile([P, T * E], F16, name=f"osb_{b}") for b in range(NB)]
    ysb = [qpool.tile([P, T * E], F32, name=f"ysb_{b}") for b in range(NB)]

    # ---------------- phase Q: quantum features --------------------------
    # The two batches' chains are emitted interleaved so the engines overlap
    # them (each work-pool tag has bufs=2, so b=0/b=1 get distinct buffers).
    MAGIC = 12582912.0  # 1.5 * 2**23
    TWO_PI = 6.283185307179586
    xsb, phb, usb = {}, {}, {}
    for b in range(NB):
        xb = x_in[b].rearrange("(p t) w -> p (t w)", p=P)
        xs = xsb[b] = work.tile([P, T * E], F32, tag="xs", name=f"xs{b}")
        nc.sync.dma_start(xs[:], xb)
    for b in range(NB):
        x3 = xsb[b].rearrange("p (t w) -> p t w", w=E)
        ph = phb[b] = work.tile([P, T * E], F32, tag="ph", name=f"ph{b}")
        p3 = ph.rearrange("p (t w) -> p t w", w=E)
        for w in range(E):
            nc.gpsimd.tensor_scalar_add(p3[:, :, w], x3[:, :, w], thp_sb[:, w : w + 1])
    for b in range(NB):
        # range-reduce ph mod 2*pi into [-pi, pi] (Sin spline domain):
        # n = round(ph / 2pi) via the fp32 magic-constant trick, ph -= n * 2pi
        ph = phb[b]
        rt = work.tile([P, T * E], F32, tag="rt")
        nc.gpsimd.tensor_scalar(
            rt[:], ph[:], 1.0 / TWO_PI, MAGIC, mybir.AluOpType.mult, mybir.AluOpType.add
        )
        nc.gpsimd.tensor_scalar(
            rt[:], rt[:], MAGIC, -TWO_PI, mybir.AluOpType.subtract, mybir.AluOpType.mult
        )
        nc.gpsimd.tensor_add(ph[:], ph[:], rt[:])
        us = usb[b] = work.tile([P, T * E], F32, tag="us", name=f"us{b}")
        nc.scalar.activation(us[:], ph[:], AF.Sin)
    for b in range(NB):
        u3 = usb[b].rearrange("p (t w) -> p t w", w=E)
        q = q9[b]
        nc.gpsimd.memset(q[:], 1.0)
        q3 = q.rearrange("p (t e) -> p t e", e=E9)
        nc.gpsimd.tensor_mul(q3[:, :, 1], u3[:, :, 0], u3[:, :, 1])
        for w in range(2, E):
            nc.gpsimd.tensor_mul(q3[:, :, w], q3[:, :, w - 1], u3[:, :, w])
        nc.gpsimd.tensor_mul(q3[:, :, 0], u3[:, :, 1], u3[:, :, 2])
        for w in range(3, E):
            nc.gpsimd.tensor_mul(q3[:, :, 0], q3[:, :, 0], u3[:, :, w])
        nc.gpsimd.tensor_copy(q9h[b][:], q[:])
    for b in range(NB):
        # transpose q9 token-chunks into qT rows 0:9 (col 128*t + p), then
        # replicate the slice to partition strips 32/64/96 via SBUF DMA
        q3 = q9[b].rearrange("p (t e) -> p t e", e=E9)
        for c0 in range(0, T, 4):
            tp = outps.tile([P, IB], F32, tag="X")
            for c in range(4):
                nc.tensor.transpose(
                    tp[0:E9, c * P : (c + 1) * P], q3[:, c0 + c, :], ident[:]
                )
            cols = slice(c0 * P, (c0 + 4) * P)
            nc.vector.tensor_copy(qT[b][0:E9, cols], tp[0:E9, :])
        for r in range(1, 4):
            nc.sync.dma_start(qT[b][32 * r : 32 * r + E, :], qT[b][0:E, :])

    # ---------------- phases A + C, batch-pipelined -----------------------
    def combine(b):
        # phase C: gather DMAs (glh row 8 stays ones for the bias), then
        # the 8x8 combine against wcb on PE.
        glh = qpool.tile([P, M4 * P * E], F16, name=f"glh_{b}")
        nc.gpsimd.memset(glh[:], 1.0)
        glh4 = glh.rearrange("p (mt pp k) -> p mt pp k", pp=P, k=E)
        og = oscr[b].rearrange("(mt pp e) w -> e mt pp w", e=E, pp=P)
        for mt in range(M4):
            nc.sync.dma_start(glh4[0:E, mt], og[:, mt])
        # serial fp16 matmuls into one PSUM bank; MM (k, mt) only needs
        # gather piece mt, so matmuls pipeline against the gather DMAs.
        # mt-major order so the first MMs depend on the first piece only.
        # alternate PSUM banks between consecutive MMs (k even/odd) so each
        # pair drains concurrently; same-bank concurrent drains are fatal.
        rp = scpA.tile([P, 2 * IB], F32, tag="scA")
        for mi in range(S // P):
            mt, k = mi // E, mi % E
            base = (k % 2) * IB + ((k // 2) * M4 + mt) * E
            nc.tensor.matmul(
                rp[:, base : base + E],
                glh4[0:E9, mt, :, k],
                wcb_sb[0:E9, :],
                start=True,
                stop=True,
            )
        # ysb[p, (k*M4+mt)*E + j] <- rp[p, (k%2)*IB + ((k//2)*M4+mt)*E + j]
        y5 = ysb[b].rearrange("p (k2 par mt j) -> p k2 par mt j", par=2, mt=M4, j=E)
        for par in range(2):
            nc.vector.tensor_copy(
                y5[:, :, par],
                rp[:, par * IB : par * IB + (T * E) // 2].rearrange(
                    "p (k2 mt j) -> p k2 mt j", mt=M4, j=E
                ),
            )
        nc.sync.dma_start(
            y[b].rearrange("(m pp) j -> pp m j", pp=P),
            ysb[b].rearrange("p (m j) -> p m j", j=E),
        )

    pending_combine = None
    for b in range(NB):
        qh3 = q9h[b].rearrange("p (t e) -> p t e", e=E9)
        o3 = osb[b].rearrange("p (t w) -> p t w", w=E)
        pending = None  # deferred normalize of the previous i-block

        def normalize(X, ib):
            # fp16 Xs halves the (fp32-rate) LDWEIGHTS+MM cost of the sel
            # MMs.  X holds sums of up to 4096 exps (<=1.9e5 worst case), so
            # scale by 1/16 into fp16 range; the factor cancels in num/den.
            Xs = work.tile([P, IB], F16, tag="Xs")
            nc.vector.tensor_scalar_mul(Xs[:], X[:], 1.0 / 16.0)
            Y = outps.tile([P, IB], F32, tag="X")
            for c in range(CPI):
                nc.tensor.matmul(
                    Y[:, c * E9 : (c + 1) * E9],
                    Xs[:, c * P : (c + 1) * P],
                    sel_sb[:],
                    start=True,
                    stop=True,
                )
            Y3 = Y[:, 0 : CPI * E9].rearrange("p (c e) -> p c e", e=E9)
            rec = work.tile([P, CPI], F32, tag="rec")
            nc.vector.reciprocal(rec[:], Y3[:, :, 8])
            for c in range(CPI):
                nc.vector.tensor_scalar_mul(
                    o3[:, ib * CPI + c, :], Y3[:, c, 0:E], rec[:, c : c + 1]
                )

        def emit_accums(Xa, g0a, gna, exA_t, exD_t, nA):
            for g in range(gna):
                tj = g0a + g
                cs = 32 * (tj % 4)
                src = exA_t[:, g * IB : (g + 1) * IB] if g < nA else (
                    exD_t[:, (g - nA) * IB : (g - nA + 1) * IB]
                )
                nc.tensor.matmul(
                    Xa[cs : cs + E9, :],
                    qh3[:, tj, :],
                    src,
                    start=(tj < 4),
                    stop=(tj >= T - 4),
                    tile_position=(0, cs),
                    skip_group_check=True,
                )

        from collections import deque

        pend_accs = deque()  # (X, g0, gn, ex, last_of_iblock, ib), lag-2
        ngroup = 0

        def pop_acc():
            nonlocal pending
            Xa, g0a, gna, exA_t, exD_t, nA, lastg, iba = pend_accs.popleft()
            emit_accums(Xa, g0a, gna, exA_t, exD_t, nA)
            if lastg:
                pending = (Xa, iba)

        for ib in range(NIB):
            X = outps.tile([P, IB], F32, tag="X")
            for g0 in range(0, T, JG):
                gn = min(JG, T - g0)
                # scores(g) first so exp(g) launches as early as possible;
                # accum(g-2) afterwards fills the PE while ACT/DVE exp(g).
                # Both are gated on exp(g-2) (the scA/scD buffer WAR), which
                # finished ~2 cadences ago, so the PE never head-blocks.
                nA = gn - 1
                scA = scpA.tile([P, 2 * IB], F32, tag="scA")
                scD = scpD.tile([P, IB], F32, tag="scD")
                for g in range(gn):
                    tj = g0 + g
                    rb = 32 * ((g0 + g) % 4)
                    dst = scA[:, g * IB : (g + 1) * IB] if g < nA else scD[:]
                    nc.tensor.matmul(
                        dst,
                        qT[b][rb : rb + E, tj * P : (tj + 1) * P],
                        qT[b][rb : rb + E, ib * IB : (ib + 1) * IB],
                        start=True,
                        stop=True,
                        tile_position=(rb, 0),
                    )
                if len(pend_accs) >= 2:
                    pop_acc()
                exA = expp.tile([P, 2 * IB], F16, tag="exA")
                exD = expp.tile([P, 2 * IB], F16, tag="exD")
                exDi = exD[:].bitcast(I16)
                nc.scalar.activation(
                    exA[:, 0 : nA * IB], scA[:, 0 : nA * IB], AF.Exp, scale=INV_SQRT2
                )
                nc.vector.tensor_scalar(
                    exDi[:, 0:IB],
                    scD[:],
                    EXP_C1,
                    EXP_C2,
                    mybir.AluOpType.mult,
                    mybir.AluOpType.add,
                )
                if pending is not None:
                    normalize(*pending)
                    pending = None
                if ngroup == 12 and pending_combine is not None:
                    combine(pending_combine)
                    pending_combine = None
                pend_accs.append((X, g0, gn, exA, exD, nA, g0 + JG >= T, ib))
                ngroup += 1
        # flush trailing accums + normalizes
        while pend_accs:
            pop_acc()
            if pending is not None and pend_accs:
                normalize(*pending)
                pending = None
        if pending is not None:
            normalize(*pending)
            pending = None
        nc.sync.dma_start(oscr[b].rearrange("(p t) w -> p (t w)", p=P), osb[b][:])
        pending_combine = b
    combine(pending_combine)


def build_nc(S=4096, NB=2):
    nc = bacc.Bacc(None, target_bir_lowering=False)
    x_in = nc.dram_tensor("x", (NB, S, E), F32, kind="ExternalInput")
    thp = nc.dram_tensor("thp", (P, E), F32, kind="ExternalInput")
    wcb = nc.dram_tensor("wcb", (P, E), F16, kind="ExternalInput")
    sel = nc.dram_tensor("sel", (P, E9), F16, kind="ExternalInput")
    y = nc.dram_tensor("y", (NB, S, E), F32, kind="ExternalOutput")
    oscr = nc.dram_tensor("oscr", (NB, S, E), F16)
    with tile.TileContext(nc) as tc:
        _body(tc, x_in[:], thp[:], wcb[:], sel[:], y[:], oscr[:], S, NB)
    nc.compile()
    return nc


def host_inputs(theta, w_combine, b_combine):
    thp = np.tile(
        (np.asarray(theta, np.float32) + np.float32(np.pi / 2))[None, :], (P, 1)
    ).astype(np.float32)
    wcb9 = np.concatenate(
        [np.asarray(w_combine, np.float32).T, np.asarray(b_combine, np.float32)[None]],
        axis=0,
    ).astype(np.float32)
    wcb = np.zeros((P, E), np.float16)
    for st in range(4):
        wcb[32 * st : 32 * st + E9] = wcb9.astype(np.float16)
    sel = np.zeros((P, E9), np.float16)
    for st in range(4):
        for e in range(E9):
            sel[32 * st + e, e] = 1.0
    return thp, wcb, sel


_NC_CACHE = {}


def kernel(x, theta, w_combine, b_combine):
    from concourse.bass_utils import run_bass_kernel_spmd

    x = np.asarray(x, np.float32)
    B, S, _ = x.shape
    NCORES = 8
    NB = B // NCORES
    key = (S, NB)
    if key not in _NC_CACHE:
        _NC_CACHE[key] = build_nc(S=S, NB=NB)
    nc = _NC_CACHE[key]
    thp, wcb, sel = host_inputs(theta, w_combine, b_combine)
    in_maps = [
        {"x": x[c * NB : (c + 1) * NB], "thp": thp, "wcb": wcb, "sel": sel}
        for c in range(NCORES)
    ]
    res = run_bass_kernel_spmd(nc, in_maps, list(range(NCORES))).results
    return np.concatenate([res[c]["y"] for c in range(NCORES)], axis=0)


# revision 33
# speedup vs baseline: 1.0822x; 1.0822x over previous
"""Trainium2 Bass kernel for nn_MultiHeadAttentionQuantum.

Math simplification (verified vs reference to ~5e-7):
  The per-token quantum feature map RX(x+theta) -> CNOT ring -> <Z_w>
  collapses to products of cosines. With u_w = cos(x_w + theta_w):
      q_0 = u1*u2*...*u7
      q_w = u0*u1*...*uw   (w = 1..7)
  Then per batch: scores = q @ q.T / sqrt(2); attn = softmax(scores);
  out = attn @ q; out' = swapaxes(out,1,2).reshape(S,8);  y = out' @ Wc.T + b.
  Softmax max-subtraction is skipped (|scores| <= 5.7, exp <= 287, safe in
  fp32). Row sums come free as a ones-column in the second matmul.

Sharding: data-parallel over batch: 16 batches -> 8 cores x 2 batches.

The v0 kernel was paced by the ACT engine: softmax needs exp of all 16.7M
scores per batch and ACT runs 1 elem/lane/cycle at 1.2 GHz (~218us/core for
33.5M exps).  This version splits each 3-chunk score group between two
engines: ACT takes 2 chunks (exact spline exp), DVE takes 1 chunk via a
bitcast-exp trick that computes the fp16 BIT PATTERN of exp(z/sqrt2)
directly in one instruction:
  bits16 = int16(z * (2^10*log2e/sqrt2) + (15*2^10 + bias))
(Schraudolph in the fp16 bit domain; max rel err ~3%, which softmax
normalization averages down to <5e-3 end-to-end because attention here is
near-uniform, eff. N ~ 2000-4000 of 4096).  The int16 result is written into
a bitcast view of the fp16 exD tile, so the accum matmul consumes it as fp16
with zero extra passes.  The Pool engine (which cannot read PSUM) takes the
SBUF-side phase-Q vector work off the DVE.

Per-core device pipeline:
  phase Q (per batch): DMA x p-major (token s = 32p + t), add theta+pi/2 per
    wire (DVE per-partition scalar), range-reduce mod 2pi, u = ACT Sin,
    13 strided DVE muls -> q9 [128, T, 9] fp32 (col 8 = ones) + fp16 copy,
    PE-transpose chunks -> qT [128, S] fp16 with the 8 feature rows
    replicated at partition strips 0/32/64/96 (for row-group packing).
  phase A (per batch, per 512-token i-block):
    scores: 3 row-group-packed K=8 fp16 matmuls per group -> PSUM [128,1536]
    exp:    3-engine split as above, PSUM->SBUF fp16
    accum:  col-group-packed matmuls X[32s:32s+9] += q9_j^T @ exp
            (strip s = chunk%4; strips summed later by the sel matmul;
            start=(tj<4) clears each strip's first accumulation, so no
            X memset is needed)
    normalize (software-pipelined one i-block behind): DVE copy X->SBUF,
    4 matmuls vs sel[128,9] (sums the 4 strips AND transposes to
    token-major), DVE reciprocal of the ones-row, DVE scale -> osb.
  phase C (per batch, deferred into the next batch's stream so it overlaps
    that batch's attention): the reference's swapaxes+reshape+combine is
    y[128m+p, j] = sum_e oscr[8*(128*mt+p)+e, k] * Wc[j,e] + b[j] with
    m = (S/1024)k + mt.  All fp16 (single-pass PE): per-mt strided gather
    DMAs into glh (row 8 = ones for the bias), 32 mt-major matmuls vs
    wcb=[Wc.T; b] into one PSUM bank (serial drains — concurrent packed
    drains into ONE bank are fatal on HW), one DVE copy, one strided store.
"""

import numpy as np

import concourse.bass as bass
import concourse.bacc as bacc
import concourse.tile as tile
from concourse import mybir
from concourse.masks import make_identity
from concourse._compat import with_exitstack

F32 = mybir.dt.float32
F16 = mybir.dt.float16
I16 = mybir.dt.int16
AF = mybir.ActivationFunctionType
P = 128
E = 8
E9 = 9
IB = 512          # i-block width (tokens per output accumulation block)
JG = 3            # j-chunks per exp group (3 PSUM banks per scores buffer)
INV_SQRT2 = 0.7071067811865476

# exp split at chunk granularity: per 3-chunk group, the first 2 chunks go
# to ACT (exact spline exp), the last to the DVE bitcast trick.  The scores
# land in SEPARATE PSUM tiles per consumer engine (scA/scD): the tile
# dependency tracker serializes multiple reader engines of one tile (it only
# records the last reader per region), which would chain DVE-exp after
# ACT-exp and pace the whole kernel.  The Pool engine cannot read PSUM, so
# it takes the SBUF-side vector work instead.

# fp16-bit-domain Schraudolph constants: bits = int16(z*EXP_C1 + EXP_C2)
EXP_C1 = float(2.0**10 * np.log2(np.e) * INV_SQRT2)
EXP_C2 = float(15 * 2.0**10 + 0.5 - 45.0)


@with_exitstack
def _body(ctx, tc, x_in, thp, wcb, sel, y, oscr, S, NB):
    nc = tc.nc
    T = S // P                 # token-chunks (tokens per partition)
    NIB = S // IB              # i-blocks per batch
    M4 = S // (P * E)          # row-tiles per combine feature block
    CPI = IB // P              # chunks per i-block (4)

    const = ctx.enter_context(tc.tile_pool(name="const", bufs=1))
    qpool = ctx.enter_context(tc.tile_pool(name="qdata", bufs=1))
    work = ctx.enter_context(tc.tile_pool(name="work", bufs=2))
    expp = ctx.enter_context(tc.tile_pool(name="expp", bufs=4))
    scpA = ctx.enter_context(tc.tile_pool(name="scpA", bufs=2, space="PSUM"))
    scpD = ctx.enter_context(tc.tile_pool(name="scpD", bufs=2, space="PSUM"))
    outps = ctx.enter_context(tc.tile_pool(name="outps", bufs=2, space="PSUM"))

    ident = const.tile([P, P], F32)
    make_identity(nc, ident[:])
    identh = const.tile([P, P], F16)
    nc.vector.tensor_copy(identh[:], ident[:])
    thp_sb = const.tile([P, E], F32)
    nc.sync.dma_start(thp_sb[:], thp[:])
    wcb_sb = const.tile([P, E], F16)
    nc.sync.dma_start(wcb_sb[:], wcb[:])
    sel_sb = const.tile([P, E9], F16)
    nc.sync.dma_start(sel_sb[:], sel[:])

    q9 = [qpool.tile([P, T * E9], F32, name=f"q9_{b}") for b in range(NB)]
    q9h = [qpool.tile([P, T * E9], F16, name=f"q9h_{b}") for b in range(NB)]
    qT = [qpool.tile([P, S], F16, name=f"qT_{b}") for b in range(NB)]
    osb = [qpool.tile([P, T * E], F16, name=f"osb_{b}") for b in range(NB)]
    ysb = [qpool.tile([P, T * E], F32, name=f"ysb_{b}") for b in range(NB)]

    # ---------------- phase Q: quantum features --------------------------
    # The two batches' chains are emitted interleaved so the engines overlap
    # them (each work-pool tag has bufs=2, so b=0/b=1 get distinct buffers).
    MAGIC = 12582912.0  # 1.5 * 2**23
    TWO_PI = 6.283185307179586
    xsb, phb, usb = {}, {}, {}
    for b in range(NB):
        xb = x_in[b].rearrange("(p t) w -> p (t w)", p=P)
        xs = xsb[b] = work.tile([P, T * E], F32, tag="xs", name=f"xs{b}")
        nc.sync.dma_start(xs[:], xb)
    for b in range(NB):
        x3 = xsb[b].rearrange("p (t w) -> p t w", w=E)
        ph = phb[b] = work.tile([P, T * E], F32, tag="ph", name=f"ph{b}")
        p3 = ph.rearrange("p (t w) -> p t w", w=E)
        for w in range(E):
            nc.vector.tensor_scalar_add(p3[:, :, w], x3[:, :, w], thp_sb[:, w : w + 1])
    for b in range(NB):
        # range-reduce ph mod 2*pi into [-pi, pi] (Sin spline domain):
        # n = round(ph / 2pi) via the fp32 magic-constant trick, ph -= n * 2pi
        ph = phb[b]
        rt = work.tile([P, T * E], F32, tag="rt")
        nc.vector.tensor_scalar(
            rt[:], ph[:], 1.0 / TWO_PI, MAGIC, mybir.AluOpType.mult, mybir.AluOpType.add
        )
        nc.vector.tensor_scalar(
            rt[:], rt[:], MAGIC, -TWO_PI, mybir.AluOpType.subtract, mybir.AluOpType.mult
        )
        nc.vector.tensor_add(ph[:], ph[:], rt[:])
        us = usb[b] = work.tile([P, T * E], F32, tag="us", name=f"us{b}")
        nc.scalar.activation(us[:], ph[:], AF.Sin)
    for b in range(NB):
        u3 = usb[b].rearrange("p (t w) -> p t w", w=E)
        q = q9[b]
        nc.vector.memset(q[:], 1.0)
        q3 = q.rearrange("p (t e) -> p t e", e=E9)
        nc.vector.tensor_mul(q3[:, :, 1], u3[:, :, 0], u3[:, :, 1])
        for w in range(2, E):
            nc.vector.tensor_mul(q3[:, :, w], q3[:, :, w - 1], u3[:, :, w])
        nc.vector.tensor_mul(q3[:, :, 0], u3[:, :, 1], u3[:, :, 2])
        for w in range(3, E):
            nc.vector.tensor_mul(q3[:, :, 0], q3[:, :, 0], u3[:, :, w])
        nc.vector.tensor_copy(q9h[b][:], q[:])
    for b in range(NB):
        # transpose q9 token-chunks into qT rows 0:9 (col 128*t + p), then
        # replicate the slice to partition strips 32/64/96 via SBUF DMA
        q3 = q9[b].rearrange("p (t e) -> p t e", e=E9)
        for c0 in range(0, T, 4):
            tp = outps.tile([P, IB], F32, tag="X", name="tp")
            for c in range(4):
                nc.tensor.transpose(
                    tp[0:E9, c * P : (c + 1) * P], q3[:, c0 + c, :], ident[:]
                )
            cols = slice(c0 * P, (c0 + 4) * P)
            nc.vector.tensor_copy(qT[b][0:E9, cols], tp[0:E9, :])
        for r in range(1, 4):
            nc.sync.dma_start(qT[b][32 * r : 32 * r + E, :], qT[b][0:E, :])

    # ---------------- phases A + C, batch-pipelined -----------------------
    def combine(b):
        # phase C: gather DMAs (glh row 8 stays ones for the bias), then
        # the 8x8 combine against wcb on PE.
        glh = qpool.tile([P, M4 * P * E], F16, name=f"glh_{b}")
        nc.gpsimd.memset(glh[:], 1.0)
        glh4 = glh.rearrange("p (mt pp k) -> p mt pp k", pp=P, k=E)
        og = oscr[b].rearrange("(mt pp e) w -> e mt pp w", e=E, pp=P)
        for mt in range(M4):
            nc.sync.dma_start(glh4[0:E, mt], og[:, mt])
        # serial fp16 matmuls into one PSUM bank; MM (k, mt) only needs
        # gather piece mt, so matmuls pipeline against the gather DMAs.
        # mt-major order so the first MMs depend on the first piece only.
        # alternate PSUM banks between consecutive MMs (k even/odd) so each
        # pair drains concurrently; same-bank concurrent drains are fatal.
        rp = scpA.tile([P, 2 * IB], F32, tag="scA")
        for mi in range(S // P):
            mt, k = mi // E, mi % E
            base = (k % 2) * IB + ((k // 2) * M4 + mt) * E
            nc.tensor.matmul(
                rp[:, base : base + E],
                glh4[0:E9, mt, :, k],
                wcb_sb[0:E9, :],
                start=True,
                stop=True,
            )
        # ysb[p, (k*M4+mt)*E + j] <- rp[p, (k%2)*IB + ((k//2)*M4+mt)*E + j]
        y5 = ysb[b].rearrange("p (k2 par mt j) -> p k2 par mt j", par=2, mt=M4, j=E)
        for par in range(2):
            nc.vector.tensor_copy(
                y5[:, :, par],
                rp[:, par * IB : par * IB + (T * E) // 2].rearrange(
                    "p (k2 mt j) -> p k2 mt j", mt=M4, j=E
                ),
            )
        nc.sync.dma_start(
            y[b].rearrange("(m pp) j -> pp m j", pp=P),
            ysb[b].rearrange("p (m j) -> p m j", j=E),
        )

    pending_combine = None
    for b in range(NB):
        qh3 = q9h[b].rearrange("p (t e) -> p t e", e=E9)
        o3 = osb[b].rearrange("p (t w) -> p t w", w=E)
        pending = None  # deferred normalize of the previous i-block

        def normalize(X, ib):
            # fp16 Xs halves the (fp32-rate) LDWEIGHTS+MM cost of the sel
            # MMs.  X holds sums of up to 4096 exps (<=1.9e5 worst case), so
            # scale by 1/16 into fp16 range; the factor cancels in num/den.
            Xs = work.tile([P, IB], F16, tag="Xs")
            nc.vector.tensor_scalar_mul(Xs[:], X[:], 1.0 / 16.0)
            Y = outps.tile([P, IB], F32, tag="X")
            for c in range(CPI):
                nc.tensor.matmul(
                    Y[:, c * E9 : (c + 1) * E9],
                    Xs[:, c * P : (c + 1) * P],
                    sel_sb[:],
                    start=True,
                    stop=True,
                )
            Y3 = Y[:, 0 : CPI * E9].rearrange("p (c e) -> p c e", e=E9)
            rec = work.tile([P, CPI], F32, tag="rec")
            nc.vector.reciprocal(rec[:], Y3[:, :, 8])
            for c in range(CPI):
                nc.vector.tensor_scalar_mul(
                    o3[:, ib * CPI + c, :], Y3[:, c, 0:E], rec[:, c : c + 1]
                )

        def emit_accums(Xa, g0a, gna, exA_t, exD_t, nA):
            for g in range(gna):
                tj = g0a + g
                cs = 32 * (tj % 4)
                src = exA_t[:, g * IB : (g + 1) * IB] if g < nA else (
                    exD_t[:, (g - nA) * IB : (g - nA + 1) * IB]
                )
                nc.tensor.matmul(
                    Xa[cs : cs + E9, :],
                    qh3[:, tj, :],
                    src,
                    start=(tj < 4),
                    stop=(tj >= T - 4),
                    tile_position=(0, cs),
                    skip_group_check=True,
                )

        from collections import deque

        pend_accs = deque()  # (X, g0, gn, ex, last_of_iblock, ib), lag-2
        ngroup = 0

        def pop_acc():
            nonlocal pending
            Xa, g0a, gna, exA_t, exD_t, nA, lastg, iba = pend_accs.popleft()
            emit_accums(Xa, g0a, gna, exA_t, exD_t, nA)
            if lastg:
                pending = (Xa, iba)

        for ib in range(NIB):
            X = outps.tile([P, IB], F32, tag="X")
            for g0 in range(0, T, JG):
                gn = min(JG, T - g0)
                # scores(g) first so exp(g) launches as early as possible;
                # accum(g-2) afterwards fills the PE while ACT/DVE exp(g).
                # Both are gated on exp(g-2) (the scA/scD buffer WAR), which
                # finished ~2 cadences ago, so the PE never head-blocks.
                nA = gn - 1
                scA = scpA.tile([P, 2 * IB], F32, tag="scA")
                scD = scpD.tile([P, IB], F32, tag="scD")
                for g in range(gn):
                    tj = g0 + g
                    rb = 32 * ((g0 + g) % 4)
                    dst = scA[:, g * IB : (g + 1) * IB] if g < nA else scD[:]
                    nc.tensor.matmul(
                        dst,
                        qT[b][rb : rb + E, tj * P : (tj + 1) * P],
                        qT[b][rb : rb + E, ib * IB : (ib + 1) * IB],
                        start=True,
                        stop=True,
                        tile_position=(rb, 0),
                    )
                if len(pend_accs) >= 2:
                    pop_acc()
                exA = expp.tile([P, 2 * IB], F16, tag="exA")
                exD = expp.tile([P, 2 * IB], F16, tag="exD")
                exDi = exD[:].bitcast(I16)
                nc.scalar.activation(
                    exA[:, 0 : nA * IB], scA[:, 0 : nA * IB], AF.Exp, scale=INV_SQRT2
                )
                nc.vector.tensor_scalar(
                    exDi[:, 0:IB],
                    scD[:],
                    EXP_C1,
                    EXP_C2,
                    mybir.AluOpType.mult,
                    mybir.AluOpType.add,
                )
                if pending is not None:
                    normalize(*pending)
                    pending = None
                if ngroup == 12 and pending_combine is not None:
                    combine(pending_combine)
                    pending_combine = None
                pend_accs.append((X, g0, gn, exA, exD, nA, g0 + JG >= T, ib))
                ngroup += 1
        # flush trailing accums + normalizes
        while pend_accs:
            pop_acc()
            if pending is not None and pend_accs:
                normalize(*pending)
                pending = None
        if pending is not None:
            normalize(*pending)
            pending = None
        nc.sync.dma_start(oscr[b].rearrange("(p t) w -> p (t w)", p=P), osb[b][:])
        pending_combine = b
    combine(pending_combine)


def build_nc(S=4096, NB=2):
    nc = bacc.Bacc(None, target_bir_lowering=False)
    x_in = nc.dram_tensor("x", (NB, S, E), F32, kind="ExternalInput")
    thp = nc.dram_tensor("thp", (P, E), F32, kind="ExternalInput")
    wcb = nc.dram_tensor("wcb", (P, E), F16, kind="ExternalInput")
    sel = nc.dram_tensor("sel", (P, E9), F16, kind="ExternalInput")
    y = nc.dram_tensor("y", (NB, S, E), F32, kind="ExternalOutput")
    oscr = nc.dram_tensor("oscr", (NB, S, E), F16)
    with tile.TileContext(nc) as tc:
        _body(tc, x_in[:], thp[:], wcb[:], sel[:], y[:], oscr[:], S, NB)
    nc.compile()
    return nc


def host_inputs(theta, w_combine, b_combine):
    thp = np.tile(
        (np.asarray(theta, np.float32) + np.float32(np.pi / 2))[None, :], (P, 1)
    ).astype(np.float32)
    wcb9 = np.concatenate(
        [np.asarray(w_combine, np.float32).T, np.asarray(b_combine, np.float32)[None]],
        axis=0,
    ).astype(np.float32)
    wcb = np.zeros((P, E), np.float16)
    for st in range(4):
        wcb[32 * st : 32 * st + E9] = wcb9.astype(np.float16)
    sel = np.zeros((P, E9), np.float16)
    for st in range(4):
        for e in range(E9):
            sel[32 * st + e, e] = 1.0
    return thp, wcb, sel


_NC_CACHE = {}


def kernel(x, theta, w_combine, b_combine):
    from concourse.bass_utils import run_bass_kernel_spmd

    x = np.asarray(x, np.float32)
    B, S, _ = x.shape
    NCORES = 8
    NB = B // NCORES
    key = (S, NB)
    if key not in _NC_CACHE:
        _NC_CACHE[key] = build_nc(S=S, NB=NB)
    nc = _NC_CACHE[key]
    thp, wcb, sel = host_inputs(theta, w_combine, b_combine)
    in_maps = [
        {"x": x[c * NB : (c + 1) * NB], "thp": thp, "wcb": wcb, "sel": sel}
        for c in range(NCORES)
    ]
    res = run_bass_kernel_spmd(nc, in_maps, list(range(NCORES))).results
    return np.concatenate([res[c]["y"] for c in range(NCORES)], axis=0)


# revision 38
# speedup vs baseline: 1.0849x; 1.0025x over previous
"""Trainium2 Bass kernel for nn_MultiHeadAttentionQuantum.

Math simplification (verified vs reference to ~5e-7):
  The per-token quantum feature map RX(x+theta) -> CNOT ring -> <Z_w>
  collapses to products of cosines. With u_w = cos(x_w + theta_w):
      q_0 = u1*u2*...*u7
      q_w = u0*u1*...*uw   (w = 1..7)
  Then per batch: scores = q @ q.T / sqrt(2); attn = softmax(scores);
  out = attn @ q; out' = swapaxes(out,1,2).reshape(S,8);  y = out' @ Wc.T + b.
  Softmax max-subtraction is skipped (|scores| <= 5.7, exp <= 287, safe in
  fp32). Row sums come free as a ones-column in the second matmul.

Sharding: data-parallel over batch: 16 batches -> 8 cores x 2 batches.

The v0 kernel was paced by the ACT engine: softmax needs exp of all 16.7M
scores per batch and ACT runs 1 elem/lane/cycle at 1.2 GHz (~218us/core for
33.5M exps).  This version splits each 3-chunk score group between two
engines: ACT takes 2 chunks (exact spline exp), DVE takes 1 chunk via a
bitcast-exp trick that computes the fp16 BIT PATTERN of exp(z/sqrt2)
directly in one instruction:
  bits16 = int16(z * (2^10*log2e/sqrt2) + (15*2^10 + bias))
(Schraudolph in the fp16 bit domain; max rel err ~3%, which softmax
normalization averages down to <5e-3 end-to-end because attention here is
near-uniform, eff. N ~ 2000-4000 of 4096).  The int16 result is written into
a bitcast view of the fp16 exD tile, so the accum matmul consumes it as fp16
with zero extra passes.  The Pool engine (which cannot read PSUM) takes the
SBUF-side phase-Q vector work off the DVE.

Per-core device pipeline:
  phase Q (per batch): DMA x p-major (token s = 32p + t), add theta+pi/2 per
    wire (DVE per-partition scalar), range-reduce mod 2pi, u = ACT Sin,
    13 strided DVE muls -> q9 [128, T, 9] fp32 (col 8 = ones) + fp16 copy,
    PE-transpose chunks -> qT [128, S] fp16 with the 8 feature rows
    replicated at partition strips 0/32/64/96 (for row-group packing).
  phase A (per batch, per 512-token i-block):
    scores: 3 row-group-packed K=8 fp16 matmuls per group -> PSUM [128,1536]
    exp:    3-engine split as above, PSUM->SBUF fp16
    accum:  col-group-packed matmuls X[32s:32s+9] += q9_j^T @ exp
            (strip s = chunk%4; strips summed later by the sel matmul;
            start=(tj<4) clears each strip's first accumulation, so no
            X memset is needed)
    normalize (software-pipelined one i-block behind): DVE copy X->SBUF,
    4 matmuls vs sel[128,9] (sums the 4 strips AND transposes to
    token-major), DVE reciprocal of the ones-row, DVE scale -> osb.
  phase C (per batch, deferred into the next batch's stream so it overlaps
    that batch's attention): the reference's swapaxes+reshape+combine is
    y[128m+p, j] = sum_e oscr[8*(128*mt+p)+e, k] * Wc[j,e] + b[j] with
    m = (S/1024)k + mt.  All fp16 (single-pass PE): per-mt strided gather
    DMAs into glh (row 8 = ones for the bias), 32 mt-major matmuls vs
    wcb=[Wc.T; b] into one PSUM bank (serial drains — concurrent packed
    drains into ONE bank are fatal on HW), one DVE copy, one strided store.
"""

import numpy as np

import concourse.bass as bass
import concourse.bacc as bacc
import concourse.tile as tile
from concourse import mybir
from concourse.masks import make_identity
from concourse._compat import with_exitstack

F32 = mybir.dt.float32
F16 = mybir.dt.float16
I16 = mybir.dt.int16
AF = mybir.ActivationFunctionType
P = 128
E = 8
E9 = 9
IB = 512          # i-block width (tokens per output accumulation block)
JG = 3            # j-chunks per exp group (3 PSUM banks per scores buffer)
INV_SQRT2 = 0.7071067811865476

# exp split at chunk granularity: per 3-chunk group, the first 2 chunks go
# to ACT (exact spline exp), the last to the DVE bitcast trick.  The scores
# land in SEPARATE PSUM tiles per consumer engine (scA/scD): the tile
# dependency tracker serializes multiple reader engines of one tile (it only
# records the last reader per region), which would chain DVE-exp after
# ACT-exp and pace the whole kernel.  The Pool engine cannot read PSUM, so
# it takes the SBUF-side vector work instead.

# fp16-bit-domain Schraudolph constants: bits = int16(z*EXP_C1 + EXP_C2)
EXP_C1 = float(2.0**10 * np.log2(np.e) * INV_SQRT2)
EXP_C2 = float(15 * 2.0**10 + 0.5 - 45.0)


@with_exitstack
def _body(ctx, tc, x_in, thp, wcb, sel, y, oscr, S, NB):
    nc = tc.nc
    T = S // P                 # token-chunks (tokens per partition)
    NIB = S // IB              # i-blocks per batch
    M4 = S // (P * E)          # row-tiles per combine feature block
    CPI = IB // P              # chunks per i-block (4)

    const = ctx.enter_context(tc.tile_pool(name="const", bufs=1))
    qpool = ctx.enter_context(tc.tile_pool(name="qdata", bufs=1))
    work = ctx.enter_context(tc.tile_pool(name="work", bufs=2))
    expp = ctx.enter_context(tc.tile_pool(name="expp", bufs=4))
    scpA = ctx.enter_context(tc.tile_pool(name="scpA", bufs=2, space="PSUM"))
    scpD = ctx.enter_context(tc.tile_pool(name="scpD", bufs=2, space="PSUM"))
    outps = ctx.enter_context(tc.tile_pool(name="outps", bufs=2, space="PSUM"))

    ident = const.tile([P, P], F32)
    make_identity(nc, ident[:])
    identh = const.tile([P, P], F16)
    nc.vector.tensor_copy(identh[:], ident[:])
    thp_sb = const.tile([P, E], F32)
    nc.sync.dma_start(thp_sb[:], thp[:])
    wcb_sb = const.tile([P, E], F16)
    nc.sync.dma_start(wcb_sb[:], wcb[:])
    sel_sb = const.tile([P, E9], F16)
    nc.sync.dma_start(sel_sb[:], sel[:])

    q9 = [qpool.tile([P, T * E9], F32, name=f"q9_{b}") for b in range(NB)]
    q9h = [qpool.tile([P, T * E9], F16, name=f"q9h_{b}") for b in range(NB)]
    qT = [qpool.tile([P, S], F16, name=f"qT_{b}") for b in range(NB)]
    osb = [qpool.tile([P, T * E], F16, name=f"osb_{b}") for b in range(NB)]
    ysb = [qpool.tile([P, T * E], F32, name=f"ysb_{b}") for b in range(NB)]

    # ---------------- phase Q: quantum features --------------------------
    # The two batches' chains are emitted interleaved so the engines overlap
    # them (each work-pool tag has bufs=2, so b=0/b=1 get distinct buffers).
    MAGIC = 12582912.0  # 1.5 * 2**23
    TWO_PI = 6.283185307179586
    xsb, phb, usb = {}, {}, {}
    for b in range(NB):
        xb = x_in[b].rearrange("(p t) w -> p (t w)", p=P)
        xs = xsb[b] = work.tile([P, T * E], F32, tag="xs", name=f"xs{b}")
        nc.sync.dma_start(xs[:], xb)
    thpb = thp_sb[:].rearrange("p (o w) -> p o w", o=1).broadcast_to([P, T, E])
    for b in range(NB):
        x3 = xsb[b].rearrange("p (t w) -> p t w", w=E)
        ph = phb[b] = work.tile([P, T * E], F32, tag="ph", name=f"ph{b}")
        p3 = ph.rearrange("p (t w) -> p t w", w=E)
        nc.vector.tensor_add(p3[:, :, :], x3[:, :, :], thpb)
    for b in range(NB):
        # range-reduce ph mod 2*pi into [-pi, pi] (Sin spline domain):
        # n = round(ph / 2pi) via the fp32 magic-constant trick, ph -= n * 2pi
        ph = phb[b]
        rt = work.tile([P, T * E], F32, tag="rt")
        nc.vector.tensor_scalar(
            rt[:], ph[:], 1.0 / TWO_PI, MAGIC, mybir.AluOpType.mult, mybir.AluOpType.add
        )
        nc.vector.tensor_scalar(
            rt[:], rt[:], MAGIC, -TWO_PI, mybir.AluOpType.subtract, mybir.AluOpType.mult
        )
        nc.vector.tensor_add(ph[:], ph[:], rt[:])
        us = usb[b] = work.tile([P, T * E], F32, tag="us", name=f"us{b}")
        nc.scalar.activation(us[:], ph[:], AF.Sin)
    for b in range(NB):
        # prefix products P_w = u0..uw (w=1..7) and S = u1..u7 via a depth-4
        # tree (the serial 13-mul chain was 3.6us of critical path per batch)
        u3 = usb[b].rearrange("p (t w) -> p t w", w=E)
        q = q9[b]
        nc.vector.memset(q[:], 1.0)
        q3 = q.rearrange("p (t e) -> p t e", e=E9)
        pr = work.tile([P, T * E], F32, tag="pr", name=f"pr{b}")
        p4 = pr.rearrange("p (t w) -> p t w", w=E)
        bb, cc, dd, ee, ff = (p4[:, :, i] for i in range(5))
        nc.vector.tensor_mul(q3[:, :, 1], u3[:, :, 0], u3[:, :, 1])   # P1
        nc.vector.tensor_mul(bb, u3[:, :, 2], u3[:, :, 3])
        nc.vector.tensor_mul(cc, u3[:, :, 4], u3[:, :, 5])
        nc.vector.tensor_mul(dd, u3[:, :, 6], u3[:, :, 7])
        nc.vector.tensor_mul(q3[:, :, 2], q3[:, :, 1], u3[:, :, 2])   # P2
        nc.vector.tensor_mul(q3[:, :, 3], q3[:, :, 1], bb)            # P3
        nc.vector.tensor_mul(ee, bb, cc)
        nc.vector.tensor_mul(q3[:, :, 4], q3[:, :, 3], u3[:, :, 4])   # P4
        nc.vector.tensor_mul(q3[:, :, 5], q3[:, :, 3], cc)            # P5
        nc.vector.tensor_mul(ff, ee, dd)
        nc.vector.tensor_mul(q3[:, :, 6], q3[:, :, 5], u3[:, :, 6])   # P6
        nc.vector.tensor_mul(q3[:, :, 7], q3[:, :, 5], dd)            # P7
        nc.vector.tensor_mul(q3[:, :, 0], u3[:, :, 1], ff)            # S
        nc.vector.tensor_copy(q9h[b][:], q[:])
    for b in range(NB):
        # transpose q9 token-chunks into qT rows 0:9 (col 128*t + p), then
        # replicate the slice to partition strips 32/64/96 via SBUF DMA
        q3 = q9[b].rearrange("p (t e) -> p t e", e=E9)
        for c0 in range(0, T, 4):
            tp = outps.tile([P, IB], F32, tag="X", name="tp")
            for c in range(4):
                nc.tensor.transpose(
                    tp[0:E9, c * P : (c + 1) * P], q3[:, c0 + c, :], ident[:]
                )
            cols = slice(c0 * P, (c0 + 4) * P)
            nc.vector.tensor_copy(qT[b][0:E9, cols], tp[0:E9, :])
        for r in range(1, 4):
            nc.sync.dma_start(qT[b][32 * r : 32 * r + E, :], qT[b][0:E, :])

    # ---------------- phases A + C, batch-pipelined -----------------------
    def combine(b):
        # phase C: gather DMAs (glh row 8 stays ones for the bias), then
        # the 8x8 combine against wcb on PE.
        glh = qpool.tile([P, M4 * P * E], F16, name=f"glh_{b}")
        nc.gpsimd.memset(glh[:], 1.0)
        glh4 = glh.rearrange("p (mt pp k) -> p mt pp k", pp=P, k=E)
        og = oscr[b].rearrange("(mt pp e) w -> e mt pp w", e=E, pp=P)
        for mt in range(M4):
            nc.sync.dma_start(glh4[0:E, mt], og[:, mt])
        # serial fp16 matmuls into one PSUM bank; MM (k, mt) only needs
        # gather piece mt, so matmuls pipeline against the gather DMAs.
        # mt-major order so the first MMs depend on the first piece only.
        # alternate PSUM banks between consecutive MMs (k even/odd) so each
        # pair drains concurrently; same-bank concurrent drains are fatal.
        rp = scpA.tile([P, 2 * IB], F32, tag="scA")
        for mi in range(S // P):
            mt, k = mi // E, mi % E
            base = (k % 2) * IB + ((k // 2) * M4 + mt) * E
            nc.tensor.matmul(
                rp[:, base : base + E],
                glh4[0:E9, mt, :, k],
                wcb_sb[0:E9, :],
                start=True,
                stop=True,
            )
        # ysb[p, (k*M4+mt)*E + j] <- rp[p, (k%2)*IB + ((k//2)*M4+mt)*E + j]
        y5 = ysb[b].rearrange("p (k2 par mt j) -> p k2 par mt j", par=2, mt=M4, j=E)
        for par in range(2):
            nc.vector.tensor_copy(
                y5[:, :, par],
                rp[:, par * IB : par * IB + (T * E) // 2].rearrange(
                    "p (k2 mt j) -> p k2 mt j", mt=M4, j=E
                ),
            )
        nc.sync.dma_start(
            y[b].rearrange("(m pp) j -> pp m j", pp=P),
            ysb[b].rearrange("p (m j) -> p m j", j=E),
        )

    pending_combine = None
    for b in range(NB):
        qh3 = q9h[b].rearrange("p (t e) -> p t e", e=E9)
        o3 = osb[b].rearrange("p (t w) -> p t w", w=E)
        pend_norm = []  # deferred normalize parts; one part per group

        def normalize(X, ib):
            # fp16 Xs halves the (fp32-rate) LDWEIGHTS+MM cost of the sel
            # MMs.  X holds sums of up to 4096 exps (<=1.9e5 worst case), so
            # scale by 1/16 into fp16 range; the factor cancels in num/den.
            # Split into two parts (consumed one per group) so the PE's sel
            # MMs don't pile onto a single group's cadence.
            st = {}

            def part1():
                Xs = work.tile([P, IB], F16, tag="Xs", name="Xs")
                nc.vector.tensor_scalar_mul(Xs[:], X[:], 1.0 / 16.0)
                Y = outps.tile([P, IB], F32, tag="X", name="Y")
                for c in range(2):
                    nc.tensor.matmul(
                        Y[:, c * E9 : (c + 1) * E9],
                        Xs[:, c * P : (c + 1) * P],
                        sel_sb[:],
                        start=True,
                        stop=True,
                    )
                st["Xs"], st["Y"] = Xs, Y

            def part2():
                Xs, Y = st["Xs"], st["Y"]
                for c in range(2, CPI):
                    nc.tensor.matmul(
                        Y[:, c * E9 : (c + 1) * E9],
                        Xs[:, c * P : (c + 1) * P],
                        sel_sb[:],
                        start=True,
                        stop=True,
                    )
                Y3 = Y[:, 0 : CPI * E9].rearrange("p (c e) -> p c e", e=E9)
                rec = work.tile([P, CPI], F32, tag="rec")
                nc.vector.reciprocal(rec[:], Y3[:, :, 8])
                for c in range(CPI):
                    nc.vector.tensor_scalar_mul(
                        o3[:, ib * CPI + c, :], Y3[:, c, 0:E], rec[:, c : c + 1]
                    )

            pend_norm.append(part1)
            pend_norm.append(part2)

        def emit_accums(Xa, g0a, gna, exA_t, exD_t, nA):
            for g in range(gna):
                tj = g0a + g
                cs = 32 * (tj % 4)
                src = exA_t[:, g * IB : (g + 1) * IB] if g < nA else (
                    exD_t[:, (g - nA) * IB : (g - nA + 1) * IB]
                )
                nc.tensor.matmul(
                    Xa[cs : cs + E9, :],
                    qh3[:, tj, :],
                    src,
                    start=(tj < 4),
                    stop=(tj >= T - 4),
                    tile_position=(0, cs),
                    skip_group_check=True,
                )

        from collections import deque

        pend_accs = deque()  # (X, g0, gn, ex, last_of_iblock, ib), lag-2
        ngroup = 0

        def pop_acc():
            Xa, g0a, gna, exA_t, exD_t, nA, lastg, iba = pend_accs.popleft()
            emit_accums(Xa, g0a, gna, exA_t, exD_t, nA)
            if lastg:
                normalize(Xa, iba)

        for ib in range(NIB):
            X = outps.tile([P, IB], F32, tag="X")
            for g0 in range(0, T, JG):
                gn = min(JG, T - g0)
                # scores(g) first so exp(g) launches as early as possible;
                # accum(g-2) afterwards fills the PE while ACT/DVE exp(g).
                # Both are gated on exp(g-2) (the scA/scD buffer WAR), which
                # finished ~2 cadences ago, so the PE never head-blocks.
                nA = gn - 1
                scA = scpA.tile([P, 2 * IB], F32, tag="scA")
                scD = scpD.tile([P, IB], F32, tag="scD")
                for g in range(gn):
                    tj = g0 + g
                    rb = 32 * ((g0 + g) % 4)
                    dst = scA[:, g * IB : (g + 1) * IB] if g < nA else scD[:]
                    nc.tensor.matmul(
                        dst,
                        qT[b][rb : rb + E, tj * P : (tj + 1) * P],
                        qT[b][rb : rb + E, ib * IB : (ib + 1) * IB],
                        start=True,
                        stop=True,
                        tile_position=(rb, 0),
                    )
                if len(pend_accs) >= 2:
                    pop_acc()
                exA = expp.tile([P, 2 * IB], F16, tag="exA")
                exD = expp.tile([P, 2 * IB], F16, tag="exD")
                exDi = exD[:].bitcast(I16)
                nc.scalar.activation(
                    exA[:, 0 : nA * IB], scA[:, 0 : nA * IB], AF.Exp, scale=INV_SQRT2
                )
                nc.vector.tensor_scalar(
                    exDi[:, 0:IB],
                    scD[:],
                    EXP_C1,
                    EXP_C2,
                    mybir.AluOpType.mult,
                    mybir.AluOpType.add,
                )
                if pend_norm:
                    pend_norm.pop(0)()
                if ngroup == 12 and pending_combine is not None:
                    combine(pending_combine)
                    pending_combine = None
                pend_accs.append((X, g0, gn, exA, exD, nA, g0 + JG >= T, ib))
                ngroup += 1
        # flush trailing accums + normalizes
        while pend_accs:
            pop_acc()
            if pend_norm:
                pend_norm.pop(0)()
        while pend_norm:
            pend_norm.pop(0)()
        nc.sync.dma_start(oscr[b].rearrange("(p t) w -> p (t w)", p=P), osb[b][:])
        pending_combine = b
    combine(pending_combine)


def build_nc(S=4096, NB=2):
    nc = bacc.Bacc(None, target_bir_lowering=False)
    x_in = nc.dram_tensor("x", (NB, S, E), F32, kind="ExternalInput")
    thp = nc.dram_tensor("thp", (P, E), F32, kind="ExternalInput")
    wcb = nc.dram_tensor("wcb", (P, E), F16, kind="ExternalInput")
    sel = nc.dram_tensor("sel", (P, E9), F16, kind="ExternalInput")
    y = nc.dram_tensor("y", (NB, S, E), F32, kind="ExternalOutput")
    oscr = nc.dram_tensor("oscr", (NB, S, E), F16)
    with tile.TileContext(nc) as tc:
        _body(tc, x_in[:], thp[:], wcb[:], sel[:], y[:], oscr[:], S, NB)
    nc.compile()
    return nc


def host_inputs(theta, w_combine, b_combine):
    thp = np.tile(
        (np.asarray(theta, np.float32) + np.float32(np.pi / 2))[None, :], (P, 1)
    ).astype(np.float32)
    wcb9 = np.concatenate(
        [np.asarray(w_combine, np.float32).T, np.asarray(b_combine, np.float32)[None]],
        axis=0,
    ).astype(np.float32)
    wcb = np.zeros((P, E), np.float16)
    for st in range(4):
        wcb[32 * st : 32 * st + E9] = wcb9.astype(np.float16)
    sel = np.zeros((P, E9), np.float16)
    for st in range(4):
        for e in range(E9):
            sel[32 * st + e, e] = 1.0
    return thp, wcb, sel


_NC_CACHE = {}


def kernel(x, theta, w_combine, b_combine):
    from concourse.bass_utils import run_bass_kernel_spmd

    x = np.asarray(x, np.float32)
    B, S, _ = x.shape
    NCORES = 8
    NB = B // NCORES
    key = (S, NB)
    if key not in _NC_CACHE:
        _NC_CACHE[key] = build_nc(S=S, NB=NB)
    nc = _NC_CACHE[key]
    thp, wcb, sel = host_inputs(theta, w_combine, b_combine)
    in_maps = [
        {"x": x[c * NB : (c + 1) * NB], "thp": thp, "wcb": wcb, "sel": sel}
        for c in range(NCORES)
    ]
    res = run_bass_kernel_spmd(nc, in_maps, list(range(NCORES))).results
    return np.concatenate([res[c]["y"] for c in range(NCORES)], axis=0)


# revision 44
# speedup vs baseline: 1.0951x; 1.0094x over previous
"""Trainium2 Bass kernel for nn_MultiHeadAttentionQuantum.

Math simplification (verified vs reference to ~5e-7):
  The per-token quantum feature map RX(x+theta) -> CNOT ring -> <Z_w>
  collapses to products of cosines. With u_w = cos(x_w + theta_w):
      q_0 = u1*u2*...*u7
      q_w = u0*u1*...*uw   (w = 1..7)
  Then per batch: scores = q @ q.T / sqrt(2); attn = softmax(scores);
  out = attn @ q; out' = swapaxes(out,1,2).reshape(S,8);  y = out' @ Wc.T + b.
  Softmax max-subtraction is skipped (|scores| <= 5.7, exp <= 287, safe in
  fp32). Row sums come free as a ones-column in the second matmul.

Sharding: data-parallel over batch: 16 batches -> 8 cores x 2 batches.

The v0 kernel was paced by the ACT engine: softmax needs exp of all 16.7M
scores per batch and ACT runs 1 elem/lane/cycle at 1.2 GHz (~218us/core for
33.5M exps).  This version splits each 3-chunk score group between two
engines: ACT takes 2 chunks (exact spline exp), DVE takes 1 chunk via a
bitcast-exp trick that computes the fp16 BIT PATTERN of exp(z/sqrt2)
directly in one instruction:
  bits16 = int16(z * (2^10*log2e/sqrt2) + (15*2^10 + bias))
(Schraudolph in the fp16 bit domain; max rel err ~3%, which softmax
normalization averages down to <5e-3 end-to-end because attention here is
near-uniform, eff. N ~ 2000-4000 of 4096).  The int16 result is written into
a bitcast view of the fp16 exD tile, so the accum matmul consumes it as fp16
with zero extra passes.  The Pool engine (which cannot read PSUM) takes the
SBUF-side phase-Q vector work off the DVE.

Per-core device pipeline:
  phase Q (per batch): DMA x p-major (token s = 32p + t), add theta+pi/2 per
    wire (DVE per-partition scalar), range-reduce mod 2pi, u = ACT Sin,
    13 strided DVE muls -> q9 [128, T, 9] fp32 (col 8 = ones) + fp16 copy,
    PE-transpose chunks -> qT [128, S] fp16 with the 8 feature rows
    replicated at partition strips 0/32/64/96 (for row-group packing).
  phase A (per batch, per 512-token i-block):
    scores: 3 row-group-packed K=8 fp16 matmuls per group -> PSUM [128,1536]
    exp:    3-engine split as above, PSUM->SBUF fp16
    accum:  col-group-packed matmuls X[32s:32s+9] += q9_j^T @ exp
            (strip s = chunk%4; strips summed later by the sel matmul;
            start=(tj<4) clears each strip's first accumulation, so no
            X memset is needed)
    normalize (software-pipelined one i-block behind): DVE copy X->SBUF,
    4 matmuls vs sel[128,9] (sums the 4 strips AND transposes to
    token-major), DVE reciprocal of the ones-row, DVE scale -> osb.
  phase C (per batch, deferred into the next batch's stream so it overlaps
    that batch's attention): the reference's swapaxes+reshape+combine is
    y[128m+p, j] = sum_e oscr[8*(128*mt+p)+e, k] * Wc[j,e] + b[j] with
    m = (S/1024)k + mt.  All fp16 (single-pass PE): per-mt strided gather
    DMAs into glh (row 8 = ones for the bias), 32 mt-major matmuls vs
    wcb=[Wc.T; b] into one PSUM bank (serial drains — concurrent packed
    drains into ONE bank are fatal on HW), one DVE copy, one strided store.
"""

import numpy as np

import concourse.bass as bass
import concourse.bacc as bacc
import concourse.tile as tile
from concourse import mybir
from concourse.masks import make_identity
from concourse._compat import with_exitstack

F32 = mybir.dt.float32
F16 = mybir.dt.float16
I16 = mybir.dt.int16
AF = mybir.ActivationFunctionType
P = 128
E = 8
E9 = 9
IB = 512          # i-block width (tokens per output accumulation block)
JG = 3            # j-chunks per exp group (3 PSUM banks per scores buffer)
INV_SQRT2 = 0.7071067811865476

# exp split at chunk granularity: per 3-chunk group, the first 2 chunks go
# to ACT (exact spline exp), the last to the DVE bitcast trick.  The scores
# land in SEPARATE PSUM tiles per consumer engine (scA/scD): the tile
# dependency tracker serializes multiple reader engines of one tile (it only
# records the last reader per region), which would chain DVE-exp after
# ACT-exp and pace the whole kernel.  The Pool engine cannot read PSUM, so
# it takes the SBUF-side vector work instead.

# fp16-bit-domain Schraudolph constants: bits = int16(z*EXP_C1 + EXP_C2)
EXP_C1 = float(2.0**10 * np.log2(np.e) * INV_SQRT2)
EXP_C2 = float(15 * 2.0**10 + 0.5 - 45.0)


@with_exitstack
def _body(ctx, tc, x_in, thp, wcb, sel, y, oscr, S, NB):
    nc = tc.nc
    T = S // P                 # token-chunks (tokens per partition)
    NIB = S // IB              # i-blocks per batch
    M4 = S // (P * E)          # row-tiles per combine feature block
    CPI = IB // P              # chunks per i-block (4)

    const = ctx.enter_context(tc.tile_pool(name="const", bufs=1))
    qpool = ctx.enter_context(tc.tile_pool(name="qdata", bufs=1))
    work = ctx.enter_context(tc.tile_pool(name="work", bufs=2))
    expp = ctx.enter_context(tc.tile_pool(name="expp", bufs=4))
    scpA = ctx.enter_context(tc.tile_pool(name="scpA", bufs=2, space="PSUM"))
    scpD = ctx.enter_context(tc.tile_pool(name="scpD", bufs=2, space="PSUM"))
    outps = ctx.enter_context(tc.tile_pool(name="outps", bufs=2, space="PSUM"))

    ident = const.tile([P, P], F32)
    make_identity(nc, ident[:])
    identh = const.tile([P, P], F16)
    nc.vector.tensor_copy(identh[:], ident[:])
    thp_sb = const.tile([P, E], F32)
    nc.sync.dma_start(thp_sb[:], thp[:])
    wcb_sb = const.tile([P, E], F16)
    nc.sync.dma_start(wcb_sb[:], wcb[:])
    sel_sb = const.tile([P, E9], F16)
    nc.sync.dma_start(sel_sb[:], sel[:])

    q9 = [qpool.tile([P, T * E9], F32, name=f"q9_{b}") for b in range(NB)]
    q9h = [qpool.tile([P, T * E9], F16, name=f"q9h_{b}") for b in range(NB)]
    qT = [qpool.tile([P, S], F16, name=f"qT_{b}") for b in range(NB)]
    osb = [qpool.tile([P, T * E], F16, name=f"osb_{b}") for b in range(NB)]
    ysb = [qpool.tile([P, T * E], F32, name=f"ysb_{b}") for b in range(NB)]

    # ---------------- phase Q: quantum features --------------------------
    # The two batches' chains are emitted interleaved so the engines overlap
    # them (each work-pool tag has bufs=2, so b=0/b=1 get distinct buffers).
    MAGIC = 12582912.0  # 1.5 * 2**23
    TWO_PI = 6.283185307179586
    xsb, phb, usb = {}, {}, {}
    for b in range(NB):
        xb = x_in[b].rearrange("(p t) w -> p (t w)", p=P)
        xs = xsb[b] = work.tile([P, T * E], F32, tag="xs", name=f"xs{b}")
        nc.sync.dma_start(xs[:], xb)
    thpb = thp_sb[:].rearrange("p (o w) -> p o w", o=1).broadcast_to([P, T, E])
    for b in range(NB):
        x3 = xsb[b].rearrange("p (t w) -> p t w", w=E)
        ph = phb[b] = work.tile([P, T * E], F32, tag="ph", name=f"ph{b}")
        p3 = ph.rearrange("p (t w) -> p t w", w=E)
        nc.vector.tensor_add(p3[:, :, :], x3[:, :, :], thpb)
    for b in range(NB):
        # range-reduce ph mod 2*pi into [-pi, pi] (Sin spline domain):
        # n = round(ph / 2pi) via the fp32 magic-constant trick, ph -= n * 2pi
        ph = phb[b]
        rt = work.tile([P, T * E], F32, tag="rt")
        nc.vector.tensor_scalar(
            rt[:], ph[:], 1.0 / TWO_PI, MAGIC, mybir.AluOpType.mult, mybir.AluOpType.add
        )
        nc.vector.tensor_scalar(
            rt[:], rt[:], MAGIC, -TWO_PI, mybir.AluOpType.subtract, mybir.AluOpType.mult
        )
        nc.vector.tensor_add(ph[:], ph[:], rt[:])
        us = usb[b] = work.tile([P, T * E], F32, tag="us", name=f"us{b}")
        nc.scalar.activation(us[:], ph[:], AF.Sin)
    for b in range(NB):
        # prefix products P_w = u0..uw (w=1..7) and S = u1..u7 via a depth-4
        # tree (the serial 13-mul chain was 3.6us of critical path per batch)
        u3 = usb[b].rearrange("p (t w) -> p t w", w=E)
        q = q9[b]
        nc.vector.memset(q[:], 1.0)
        q3 = q.rearrange("p (t e) -> p t e", e=E9)
        pr = work.tile([P, T * E], F32, tag="pr", name=f"pr{b}")
        p4 = pr.rearrange("p (t w) -> p t w", w=E)
        bb, cc, dd, ee, ff = (p4[:, :, i] for i in range(5))
        nc.vector.tensor_mul(q3[:, :, 1], u3[:, :, 0], u3[:, :, 1])   # P1
        nc.vector.tensor_mul(bb, u3[:, :, 2], u3[:, :, 3])
        nc.vector.tensor_mul(cc, u3[:, :, 4], u3[:, :, 5])
        nc.vector.tensor_mul(dd, u3[:, :, 6], u3[:, :, 7])
        nc.vector.tensor_mul(q3[:, :, 2], q3[:, :, 1], u3[:, :, 2])   # P2
        nc.vector.tensor_mul(q3[:, :, 3], q3[:, :, 1], bb)            # P3
        nc.vector.tensor_mul(ee, bb, cc)
        nc.vector.tensor_mul(q3[:, :, 4], q3[:, :, 3], u3[:, :, 4])   # P4
        nc.vector.tensor_mul(q3[:, :, 5], q3[:, :, 3], cc)            # P5
        nc.vector.tensor_mul(ff, ee, dd)
        nc.vector.tensor_mul(q3[:, :, 6], q3[:, :, 5], u3[:, :, 6])   # P6
        nc.vector.tensor_mul(q3[:, :, 7], q3[:, :, 5], dd)            # P7
        nc.vector.tensor_mul(q3[:, :, 0], u3[:, :, 1], ff)            # S
        nc.vector.tensor_copy(q9h[b][:], q[:])
    # transpose q9 token-chunks into qT rows 0:9 (col 128*t + p), then
    # replicate the slice to partition strips 32/64/96 via SBUF DMA
    def emit_tp_block(bb, c0, dst):
        q3l = q9[bb].rearrange("p (t e) -> p t e", e=E9)
        for c in range(4):
            nc.tensor.transpose(
                dst[0:E9, c * P : (c + 1) * P], q3l[:, c0 + c, :], ident[:]
            )
        cols = slice(c0 * P, (c0 + 4) * P)
        nc.vector.tensor_copy(qT[bb][0:E9, cols], dst[0:E9, :])
        for r in range(1, 4):
            nc.sync.dma_start(qT[bb][32 * r : 32 * r + E, cols], qT[bb][0:E, cols])

    for b in range(NB):
        for c0 in range(0, T, 4):
            tp = outps.tile([P, IB], F32, tag="X", name="tp")
            emit_tp_block(b, c0, tp)

    # ---------------- phases A + C, batch-pipelined -----------------------
    def combine(b):
        # phase C: gather DMAs (glh row 8 stays ones for the bias), then
        # the 8x8 combine against wcb on PE.
        glh = qpool.tile([P, M4 * P * E], F16, name=f"glh_{b}")
        nc.gpsimd.memset(glh[:], 1.0)
        glh4 = glh.rearrange("p (mt pp k) -> p mt pp k", pp=P, k=E)
        og = oscr[b].rearrange("(mt pp e) w -> e mt pp w", e=E, pp=P)
        for mt in range(M4):
            nc.sync.dma_start(glh4[0:E, mt], og[:, mt])
        # serial fp16 matmuls into one PSUM bank; MM (k, mt) only needs
        # gather piece mt, so matmuls pipeline against the gather DMAs.
        # mt-major order so the first MMs depend on the first piece only.
        # alternate PSUM banks between consecutive MMs (k even/odd) so each
        # pair drains concurrently; same-bank concurrent drains are fatal.
        rp = scpA.tile([P, 2 * IB], F32, tag="scA")
        for mi in range(S // P):
            mt, k = mi // E, mi % E
            base = (k % 2) * IB + ((k // 2) * M4 + mt) * E
            nc.tensor.matmul(
                rp[:, base : base + E],
                glh4[0:E9, mt, :, k],
                wcb_sb[0:E9, :],
                start=True,
                stop=True,
            )
        # ysb[p, (k*M4+mt)*E + j] <- rp[p, (k%2)*IB + ((k//2)*M4+mt)*E + j]
        y5 = ysb[b].rearrange("p (k2 par mt j) -> p k2 par mt j", par=2, mt=M4, j=E)
        for par in range(2):
            nc.vector.tensor_copy(
                y5[:, :, par],
                rp[:, par * IB : par * IB + (T * E) // 2].rearrange(
                    "p (k2 mt j) -> p k2 mt j", mt=M4, j=E
                ),
            )
        nc.sync.dma_start(
            y[b].rearrange("(m pp) j -> pp m j", pp=P),
            ysb[b].rearrange("p (m j) -> p m j", j=E),
        )

    pending_combine = None
    for b in range(NB):
        qh3 = q9h[b].rearrange("p (t e) -> p t e", e=E9)
        o3 = osb[b].rearrange("p (t w) -> p t w", w=E)
        pend_norm = []  # deferred normalize; consumed one per group

        def normalize(X, ib):
            # fp16 Xs halves the (fp32-rate) LDWEIGHTS+MM cost of the sel
            # MMs.  X holds sums of up to 4096 exps (<=1.9e5 worst case), so
            # scale by 1/16 into fp16 range; the factor cancels in num/den.
            def run():
                Xs = work.tile([P, IB], F16, tag="Xs", name="Xs")
                nc.vector.tensor_scalar_mul(Xs[:], X[:], 1.0 / 16.0)
                Y = outps.tile([P, IB], F32, tag="X", name="Y")
                for c in range(CPI):
                    nc.tensor.matmul(
                        Y[:, c * E9 : (c + 1) * E9],
                        Xs[:, c * P : (c + 1) * P],
                        sel_sb[:],
                        start=True,
                        stop=True,
                    )
                Y3 = Y[:, 0 : CPI * E9].rearrange("p (c e) -> p c e", e=E9)
                rec = work.tile([P, CPI], F32, tag="rec")
                nc.vector.reciprocal(rec[:], Y3[:, :, 8])
                for c in range(CPI):
                    nc.vector.tensor_scalar_mul(
                        o3[:, ib * CPI + c, :], Y3[:, c, 0:E], rec[:, c : c + 1]
                    )

            pend_norm.append(run)

        def emit_accums(Xa, g0a, gna, exA_t, exD_t, nA):
            for g in range(gna):
                tj = g0a + g
                cs = 32 * (tj % 4)
                src = exA_t[:, g * IB : (g + 1) * IB] if g < nA else (
                    exD_t[:, (g - nA) * IB : (g - nA + 1) * IB]
                )
                nc.tensor.matmul(
                    Xa[cs : cs + E9, :],
                    qh3[:, tj, :],
                    src,
                    start=(tj < 4),
                    stop=(tj >= T - 4),
                    tile_position=(0, cs),
                    skip_group_check=True,
                )

        from collections import deque

        pend_accs = deque()  # (X, g0, gn, ex, last_of_iblock, ib), lag-2
        ngroup = 0

        def pop_acc():
            Xa, g0a, gna, exA_t, exD_t, nA, lastg, iba = pend_accs.popleft()
            emit_accums(Xa, g0a, gna, exA_t, exD_t, nA)
            if lastg:
                normalize(Xa, iba)

        for ib in range(NIB):
            X = outps.tile([P, IB], F32, tag="X")
            for g0 in range(0, T, JG):
                gn = min(JG, T - g0)
                # scores(g) first so exp(g) launches as early as possible;
                # accum(g-2) afterwards fills the PE while ACT/DVE exp(g).
                # Both are gated on exp(g-2) (the scA/scD buffer WAR), which
                # finished ~2 cadences ago, so the PE never head-blocks.
                nA = gn - 1
                scA = scpA.tile([P, 2 * IB], F32, tag="scA")
                scD = scpD.tile([P, IB], F32, tag="scD")
                for g in range(gn):
                    tj = g0 + g
                    rb = 32 * ((g0 + g) % 4)
                    dst = scA[:, g * IB : (g + 1) * IB] if g < nA else scD[:]
                    nc.tensor.matmul(
                        dst,
                        qT[b][rb : rb + E, tj * P : (tj + 1) * P],
                        qT[b][rb : rb + E, ib * IB : (ib + 1) * IB],
                        start=True,
                        stop=True,
                        tile_position=(rb, 0),
                    )
                if len(pend_accs) >= 2:
                    pop_acc()
                exA = expp.tile([P, 2 * IB], F16, tag="exA")
                exD = expp.tile([P, 2 * IB], F16, tag="exD")
                exDi = exD[:].bitcast(I16)
                nc.scalar.activation(
                    exA[:, 0 : nA * IB], scA[:, 0 : nA * IB], AF.Exp, scale=INV_SQRT2
                )
                nc.vector.tensor_scalar(
                    exDi[:, 0:IB],
                    scD[:],
                    EXP_C1,
                    EXP_C2,
                    mybir.AluOpType.mult,
                    mybir.AluOpType.add,
                )
                if pend_norm:
                    pend_norm.pop(0)()
                if ngroup == 12 and pending_combine is not None:
                    combine(pending_combine)
                    pending_combine = None
                pend_accs.append((X, g0, gn, exA, exD, nA, g0 + JG >= T, ib))
                ngroup += 1
        # flush trailing accums + normalizes
        while pend_accs:
            pop_acc()
            if pend_norm:
                pend_norm.pop(0)()
        while pend_norm:
            pend_norm.pop(0)()
        nc.sync.dma_start(oscr[b].rearrange("(p t) w -> p (t w)", p=P), osb[b][:])
        pending_combine = b
    combine(pending_combine)


def build_nc(S=4096, NB=2):
    nc = bacc.Bacc(None, target_bir_lowering=False)
    x_in = nc.dram_tensor("x", (NB, S, E), F32, kind="ExternalInput")
    thp = nc.dram_tensor("thp", (P, E), F32, kind="ExternalInput")
    wcb = nc.dram_tensor("wcb", (P, E), F16, kind="ExternalInput")
    sel = nc.dram_tensor("sel", (P, E9), F16, kind="ExternalInput")
    y = nc.dram_tensor("y", (NB, S, E), F32, kind="ExternalOutput")
    oscr = nc.dram_tensor("oscr", (NB, S, E), F16)
    with tile.TileContext(nc) as tc:
        _body(tc, x_in[:], thp[:], wcb[:], sel[:], y[:], oscr[:], S, NB)
    nc.compile()
    return nc


def host_inputs(theta, w_combine, b_combine):
    thp = np.tile(
        (np.asarray(theta, np.float32) + np.float32(np.pi / 2))[None, :], (P, 1)
    ).astype(np.float32)
    wcb9 = np.concatenate(
        [np.asarray(w_combine, np.float32).T, np.asarray(b_combine, np.float32)[None]],
        axis=0,
    ).astype(np.float32)
    wcb = np.zeros((P, E), np.float16)
    for st in range(4):
        wcb[32 * st : 32 * st + E9] = wcb9.astype(np.float16)
    sel = np.zeros((P, E9), np.float16)
    for st in range(4):
        for e in range(E9):
            sel[32 * st + e, e] = 1.0
    return thp, wcb, sel


_NC_CACHE = {}


def kernel(x, theta, w_combine, b_combine):
    from concourse.bass_utils import run_bass_kernel_spmd

    x = np.asarray(x, np.float32)
    B, S, _ = x.shape
    NCORES = 8
    NB = B // NCORES
    key = (S, NB)
    if key not in _NC_CACHE:
        _NC_CACHE[key] = build_nc(S=S, NB=NB)
    nc = _NC_CACHE[key]
    thp, wcb, sel = host_inputs(theta, w_combine, b_combine)
    in_maps = [
        {"x": x[c * NB : (c + 1) * NB], "thp": thp, "wcb": wcb, "sel": sel}
        for c in range(NCORES)
    ]
    res = run_bass_kernel_spmd(nc, in_maps, list(range(NCORES))).results
    return np.concatenate([res[c]["y"] for c in range(NCORES)], axis=0)


# revision 45
# speedup vs baseline: 1.1188x; 1.0216x over previous
"""Trainium2 Bass kernel for nn_MultiHeadAttentionQuantum.

Math simplification (verified vs reference to ~5e-7):
  The per-token quantum feature map RX(x+theta) -> CNOT ring -> <Z_w>
  collapses to products of cosines. With u_w = cos(x_w + theta_w):
      q_0 = u1*u2*...*u7
      q_w = u0*u1*...*uw   (w = 1..7)
  Then per batch: scores = q @ q.T / sqrt(2); attn = softmax(scores);
  out = attn @ q; out' = swapaxes(out,1,2).reshape(S,8);  y = out' @ Wc.T + b.
  Softmax max-subtraction is skipped (|scores| <= 5.7, exp <= 287, safe in
  fp32). Row sums come free as a ones-column in the second matmul.

Sharding: data-parallel over batch: 16 batches -> 8 cores x 2 batches.

The v0 kernel was paced by the ACT engine: softmax needs exp of all 16.7M
scores per batch and ACT runs 1 elem/lane/cycle at 1.2 GHz (~218us/core for
33.5M exps).  This version splits each 3-chunk score group between two
engines: ACT takes 2 chunks (exact spline exp), DVE takes 1 chunk via a
bitcast-exp trick that computes the fp16 BIT PATTERN of exp(z/sqrt2)
directly in one instruction:
  bits16 = int16(z * (2^10*log2e/sqrt2) + (15*2^10 + bias))
(Schraudolph in the fp16 bit domain; max rel err ~3%, which softmax
normalization averages down to <5e-3 end-to-end because attention here is
near-uniform, eff. N ~ 2000-4000 of 4096).  The int16 result is written into
a bitcast view of the fp16 exD tile, so the accum matmul consumes it as fp16
with zero extra passes.  The Pool engine (which cannot read PSUM) takes the
SBUF-side phase-Q vector work off the DVE.

Per-core device pipeline (both batches' phase Q emitted interleaved):
  phase Q: DMA x p-major (token s = 32p + t), add theta+pi/2 (one DVE
    broadcast add), range-reduce mod 2pi, u = ACT Sin, depth-4 product tree
    -> q9 [128, T, 9] fp32 (col 8 = ones) + fp16 copy, PE-transpose chunks
    -> qT [128, S] fp16 with the 8 feature rows replicated at partition
    strips 0/32/64/96 (for row-group packing).
  phase A (per batch, per 512-token i-block, per 3-chunk group):
    scores: 3 row-group-packed K=8 fp16 matmuls (2 into scA, 1 into scD)
    exp:    ACT exp(scA) and DVE bitcast-trick(scD) run CONCURRENTLY
    accum (emitted with a 2-group lag, after the next group's scores, so
    the in-order PE never waits on a fresh exp semaphore): col-group-packed
    matmuls X[32s:32s+9] += q9_j^T @ exp (strip s = chunk%4; strips summed
    by the sel matmul; start=(tj<4) clears each strip's first accumulation)
    normalize (pipelined one i-block behind): DVE scale X*(1/16)->fp16 Xs,
    4 fp16 matmuls vs sel[128,9] (sum the 4 strips AND transpose to
    token-major; the 1/16 cancels in num/den), DVE reciprocal, scale -> osb.
  phase C (per batch, deferred into the next batch's stream so it overlaps
    that batch's attention): the reference's swapaxes+reshape+combine is
    y[128m+p, j] = sum_e oscr[8*(128*mt+p)+e, k] * Wc[j,e] + b[j] with
    m = (S/1024)k + mt.  All fp16 (single-pass PE): per-mt strided gather
    DMAs into glh (row 8 stays ones for the bias), 32 mt-major matmuls vs
    wcb=[Wc.T; b] alternating between two PSUM banks (concurrent drains
    into one bank at the same partitions are fatal), 2 DVE copies, store.
"""

import numpy as np

import concourse.bass as bass
import concourse.bacc as bacc
import concourse.tile as tile
from concourse import mybir
from concourse.masks import make_identity
from concourse._compat import with_exitstack

F32 = mybir.dt.float32
F16 = mybir.dt.float16
I16 = mybir.dt.int16
AF = mybir.ActivationFunctionType
P = 128
E = 8
E9 = 9
IB = 512          # i-block width (tokens per output accumulation block)
JG = 3            # j-chunks per exp group (3 PSUM banks per scores buffer)
INV_SQRT2 = 0.7071067811865476

# exp split at chunk granularity: per 3-chunk group, the first 2 chunks go
# to ACT (exact spline exp), the last to the DVE bitcast trick.  The scores
# land in SEPARATE PSUM tiles per consumer engine (scA/scD): the tile
# dependency tracker serializes multiple reader engines of one tile (it only
# records the last reader per region), which would chain DVE-exp after
# ACT-exp and pace the whole kernel.  The Pool engine cannot read PSUM, so
# it takes the SBUF-side vector work instead.

# fp16-bit-domain Schraudolph constants: bits = int16(z*EXP_C1 + EXP_C2)
EXP_C1 = float(2.0**10 * np.log2(np.e) * INV_SQRT2)
EXP_C2 = float(15 * 2.0**10 + 0.5 - 45.0)


@with_exitstack
def _body(ctx, tc, x_in, thp, wcb, sel, y, oscr, S, NB):
    nc = tc.nc
    T = S // P                 # token-chunks (tokens per partition)
    NIB = S // IB              # i-blocks per batch
    M4 = S // (P * E)          # row-tiles per combine feature block
    CPI = IB // P              # chunks per i-block (4)

    const = ctx.enter_context(tc.tile_pool(name="const", bufs=1))
    qpool = ctx.enter_context(tc.tile_pool(name="qdata", bufs=1))
    work = ctx.enter_context(tc.tile_pool(name="work", bufs=2))
    expp = ctx.enter_context(tc.tile_pool(name="expp", bufs=4))
    scpA = ctx.enter_context(tc.tile_pool(name="scpA", bufs=2, space="PSUM"))
    scpD = ctx.enter_context(tc.tile_pool(name="scpD", bufs=2, space="PSUM"))
    outps = ctx.enter_context(tc.tile_pool(name="outps", bufs=2, space="PSUM"))

    ident = const.tile([P, P], F32)
    make_identity(nc, ident[:])
    identh = const.tile([P, P], F16)
    nc.vector.tensor_copy(identh[:], ident[:])
    thp_sb = const.tile([P, E], F32)
    nc.sync.dma_start(thp_sb[:], thp[:])
    wcb_sb = const.tile([P, E], F16)
    nc.sync.dma_start(wcb_sb[:], wcb[:])
    sel_sb = const.tile([P, E9], F16)
    nc.sync.dma_start(sel_sb[:], sel[:])

    q9 = [qpool.tile([P, T * E9], F32, name=f"q9_{b}") for b in range(NB)]
    q9h = [qpool.tile([P, T * E9], F16, name=f"q9h_{b}") for b in range(NB)]
    qT = [qpool.tile([P, S], F16, name=f"qT_{b}") for b in range(NB)]
    osb = [qpool.tile([P, T * E], F16, name=f"osb_{b}") for b in range(NB)]
    ysb = [qpool.tile([P, T * E], F32, name=f"ysb_{b}") for b in range(NB)]

    # ---------------- phase Q: quantum features --------------------------
    # The two batches' chains are emitted interleaved so the engines overlap
    # them (each work-pool tag has bufs=2, so b=0/b=1 get distinct buffers).
    MAGIC = 12582912.0  # 1.5 * 2**23
    TWO_PI = 6.283185307179586
    xsb, phb, usb = {}, {}, {}
    for b in range(NB):
        xb = x_in[b].rearrange("(p t) w -> p (t w)", p=P)
        xs = xsb[b] = work.tile([P, T * E], F32, tag="xs", name=f"xs{b}")
        nc.sync.dma_start(xs[:], xb)
    thpb = thp_sb[:].rearrange("p (o w) -> p o w", o=1).broadcast_to([P, T, E])
    for b in range(NB):
        x3 = xsb[b].rearrange("p (t w) -> p t w", w=E)
        ph = phb[b] = work.tile([P, T * E], F32, tag="ph", name=f"ph{b}")
        p3 = ph.rearrange("p (t w) -> p t w", w=E)
        nc.vector.tensor_add(p3[:, :, :], x3[:, :, :], thpb)
    for b in range(NB):
        # range-reduce ph mod 2*pi into [-pi, pi] (Sin spline domain):
        # n = round(ph / 2pi) via the fp32 magic-constant trick, ph -= n * 2pi
        ph = phb[b]
        rt = work.tile([P, T * E], F32, tag="rt")
        nc.vector.tensor_scalar(
            rt[:], ph[:], 1.0 / TWO_PI, MAGIC, mybir.AluOpType.mult, mybir.AluOpType.add
        )
        nc.vector.tensor_scalar(
            rt[:], rt[:], MAGIC, -TWO_PI, mybir.AluOpType.subtract, mybir.AluOpType.mult
        )
        nc.vector.tensor_add(ph[:], ph[:], rt[:])
        us = usb[b] = work.tile([P, T * E], F32, tag="us", name=f"us{b}")
        nc.scalar.activation(us[:], ph[:], AF.Sin)
    for b in range(NB):
        # prefix products P_w = u0..uw (w=1..7) and S = u1..u7 via a depth-4
        # tree (the serial 13-mul chain was 3.6us of critical path per batch)
        u3 = usb[b].rearrange("p (t w) -> p t w", w=E)
        q = q9[b]
        nc.vector.memset(q[:], 1.0)
        q3 = q.rearrange("p (t e) -> p t e", e=E9)
        pr = work.tile([P, T * E], F32, tag="pr", name=f"pr{b}")
        p4 = pr.rearrange("p (t w) -> p t w", w=E)
        bb, cc, dd, ee, ff = (p4[:, :, i] for i in range(5))
        nc.vector.tensor_mul(q3[:, :, 1], u3[:, :, 0], u3[:, :, 1])   # P1
        nc.vector.tensor_mul(bb, u3[:, :, 2], u3[:, :, 3])
        nc.vector.tensor_mul(cc, u3[:, :, 4], u3[:, :, 5])
        nc.vector.tensor_mul(dd, u3[:, :, 6], u3[:, :, 7])
        nc.vector.tensor_mul(q3[:, :, 2], q3[:, :, 1], u3[:, :, 2])   # P2
        nc.vector.tensor_mul(q3[:, :, 3], q3[:, :, 1], bb)            # P3
        nc.vector.tensor_mul(ee, bb, cc)
        nc.vector.tensor_mul(q3[:, :, 4], q3[:, :, 3], u3[:, :, 4])   # P4
        nc.vector.tensor_mul(q3[:, :, 5], q3[:, :, 3], cc)            # P5
        nc.vector.tensor_mul(ff, ee, dd)
        nc.vector.tensor_mul(q3[:, :, 6], q3[:, :, 5], u3[:, :, 6])   # P6
        nc.vector.tensor_mul(q3[:, :, 7], q3[:, :, 5], dd)            # P7
        nc.vector.tensor_mul(q3[:, :, 0], u3[:, :, 1], ff)            # S
        nc.vector.tensor_copy(q9h[b][:], q[:])
    # transpose q9 token-chunks into qT rows 0:9 (col 128*t + p), then
    # replicate the slice to partition strips 32/64/96 via SBUF DMA
    def emit_tp_block(bb, c0, dst):
        q3l = q9[bb].rearrange("p (t e) -> p t e", e=E9)
        for c in range(4):
            nc.tensor.transpose(
                dst[0:E9, c * P : (c + 1) * P], q3l[:, c0 + c, :], ident[:]
            )
        cols = slice(c0 * P, (c0 + 4) * P)
        nc.vector.tensor_copy(qT[bb][0:E9, cols], dst[0:E9, :])
        for r in range(1, 4):
            nc.sync.dma_start(qT[bb][32 * r : 32 * r + E, cols], qT[bb][0:E, cols])

    for b in range(NB):
        for c0 in range(0, T, 4):
            tp = outps.tile([P, IB], F32, tag="X", name="tp")
            emit_tp_block(b, c0, tp)

    # ---------------- phases A + C, batch-pipelined -----------------------
    def combine(b):
        # phase C: gather DMAs (glh row 8 stays ones for the bias), then
        # the 8x8 combine against wcb on PE.
        glh = qpool.tile([P, M4 * P * E], F16, name=f"glh_{b}")
        nc.gpsimd.memset(glh[:], 1.0)
        glh4 = glh.rearrange("p (mt pp k) -> p mt pp k", pp=P, k=E)
        og = oscr[b].rearrange("(mt pp e) w -> e mt pp w", e=E, pp=P)
        for mt in range(M4):
            nc.sync.dma_start(glh4[0:E, mt], og[:, mt])
        # serial fp16 matmuls into one PSUM bank; MM (k, mt) only needs
        # gather piece mt, so matmuls pipeline against the gather DMAs.
        # mt-major order so the first MMs depend on the first piece only.
        # alternate PSUM banks between consecutive MMs (k even/odd) so each
        # pair drains concurrently; same-bank concurrent drains are fatal.
        rp = scpA.tile([P, 2 * IB], F32, tag="scA")
        for mi in range(S // P):
            mt, k = mi // E, mi % E
            base = (k % 2) * IB + ((k // 2) * M4 + mt) * E
            nc.tensor.matmul(
                rp[:, base : base + E],
                glh4[0:E9, mt, :, k],
                wcb_sb[0:E9, :],
                start=True,
                stop=True,
            )
        # ysb[p, (k*M4+mt)*E + j] <- rp[p, (k%2)*IB + ((k//2)*M4+mt)*E + j]
        y5 = ysb[b].rearrange("p (k2 par mt j) -> p k2 par mt j", par=2, mt=M4, j=E)
        for par in range(2):
            nc.vector.tensor_copy(
                y5[:, :, par],
                rp[:, par * IB : par * IB + (T * E) // 2].rearrange(
                    "p (k2 mt j) -> p k2 mt j", mt=M4, j=E
                ),
            )
        nc.sync.dma_start(
            y[b].rearrange("(m pp) j -> pp m j", pp=P),
            ysb[b].rearrange("p (m j) -> p m j", j=E),
        )

    pending_combine = None
    for b in range(NB):
        qh3 = q9h[b].rearrange("p (t e) -> p t e", e=E9)
        o3 = osb[b].rearrange("p (t w) -> p t w", w=E)
        pend_norm = []  # deferred normalize; consumed one per group

        def normalize(X, ib):
            # fp16 Xs halves the (fp32-rate) LDWEIGHTS+MM cost of the sel
            # MMs.  X holds sums of up to 4096 exps (<=1.9e5 worst case), so
            # scale by 1/16 into fp16 range; the factor cancels in num/den.
            def run():
                Xs = work.tile([P, IB], F16, tag="Xs", name="Xs")
                nc.vector.tensor_scalar_mul(Xs[:], X[:], 1.0 / 16.0)
                Y = outps.tile([P, IB], F32, tag="X", name="Y")
                for c in range(CPI):
                    nc.tensor.matmul(
                        Y[:, c * E9 : (c + 1) * E9],
                        Xs[:, c * P : (c + 1) * P],
                        sel_sb[:],
                        start=True,
                        stop=True,
                    )
                Y3 = Y[:, 0 : CPI * E9].rearrange("p (c e) -> p c e", e=E9)
                rec = work.tile([P, CPI], F32, tag="rec")
                nc.vector.reciprocal(rec[:], Y3[:, :, 8])
                for c in range(CPI):
                    nc.vector.tensor_scalar_mul(
                        o3[:, ib * CPI + c, :], Y3[:, c, 0:E], rec[:, c : c + 1]
                    )

            pend_norm.append(run)

        def emit_accums(Xa, g0a, gna, exA_t, exD_t, nA):
            for g in range(gna):
                tj = g0a + g
                cs = 32 * (tj % 4)
                src = exA_t[:, g * IB : (g + 1) * IB] if g < nA else (
                    exD_t[:, (g - nA) * IB : (g - nA + 1) * IB]
                )
                nc.tensor.matmul(
                    Xa[cs : cs + E9, :],
                    qh3[:, tj, :],
                    src,
                    start=(tj < 4),
                    stop=(tj >= T - 4),
                    tile_position=(0, cs),
                    skip_group_check=True,
                )

        from collections import deque

        pend_accs = deque()  # (X, g0, gn, ex, last_of_iblock, ib), lag-2
        ngroup = 0

        def pop_acc():
            Xa, g0a, gna, exA_t, exD_t, nA, lastg, iba = pend_accs.popleft()
            emit_accums(Xa, g0a, gna, exA_t, exD_t, nA)
            if lastg:
                normalize(Xa, iba)

        for ib in range(NIB):
            X = outps.tile([P, IB], F32, tag="X")
            for g0 in range(0, T, JG):
                gn = min(JG, T - g0)
                # scores(g) first so exp(g) launches as early as possible;
                # accum(g-2) afterwards fills the PE while ACT/DVE exp(g).
                # Both are gated on exp(g-2) (the scA/scD buffer WAR), which
                # finished ~2 cadences ago, so the PE never head-blocks.
                nA = gn - 1
                scA = scpA.tile([P, 2 * IB], F32, tag="scA")
                scD = scpD.tile([P, IB], F32, tag="scD")
                for g in range(gn):
                    tj = g0 + g
                    rb = 32 * ((g0 + g) % 4)
                    dst = scA[:, g * IB : (g + 1) * IB] if g < nA else scD[:]
                    nc.tensor.matmul(
                        dst,
                        qT[b][rb : rb + E, tj * P : (tj + 1) * P],
                        qT[b][rb : rb + E, ib * IB : (ib + 1) * IB],
                        start=True,
                        stop=True,
                        tile_position=(rb, 0),
                    )
                if len(pend_accs) >= 2:
                    pop_acc()
                exA = expp.tile([P, 2 * IB], F16, tag="exA")
                exD = expp.tile([P, 2 * IB], F16, tag="exD")
                exDi = exD[:].bitcast(I16)
                nc.scalar.activation(
                    exA[:, 0 : nA * IB], scA[:, 0 : nA * IB], AF.Exp, scale=INV_SQRT2
                )
                nc.vector.tensor_scalar(
                    exDi[:, 0:IB],
                    scD[:],
                    EXP_C1,
                    EXP_C2,
                    mybir.AluOpType.mult,
                    mybir.AluOpType.add,
                )
                if pend_norm:
                    pend_norm.pop(0)()
                if ngroup == 12 and pending_combine is not None:
                    combine(pending_combine)
                    pending_combine = None
                pend_accs.append((X, g0, gn, exA, exD, nA, g0 + JG >= T, ib))
                ngroup += 1
        # flush trailing accums + normalizes
        while pend_accs:
            pop_acc()
            if pend_norm:
                pend_norm.pop(0)()
        while pend_norm:
            pend_norm.pop(0)()
        nc.sync.dma_start(oscr[b].rearrange("(p t) w -> p (t w)", p=P), osb[b][:])
        pending_combine = b
    combine(pending_combine)


def build_nc(S=4096, NB=2):
    nc = bacc.Bacc(None, target_bir_lowering=False)
    x_in = nc.dram_tensor("x", (NB, S, E), F32, kind="ExternalInput")
    thp = nc.dram_tensor("thp", (P, E), F32, kind="ExternalInput")
    wcb = nc.dram_tensor("wcb", (P, E), F16, kind="ExternalInput")
    sel = nc.dram_tensor("sel", (P, E9), F16, kind="ExternalInput")
    y = nc.dram_tensor("y", (NB, S, E), F32, kind="ExternalOutput")
    oscr = nc.dram_tensor("oscr", (NB, S, E), F16)
    with tile.TileContext(nc) as tc:
        _body(tc, x_in[:], thp[:], wcb[:], sel[:], y[:], oscr[:], S, NB)
    nc.compile()
    return nc


def host_inputs(theta, w_combine, b_combine):
    thp = np.tile(
        (np.asarray(theta, np.float32) + np.float32(np.pi / 2))[None, :], (P, 1)
    ).astype(np.float32)
    wcb9 = np.concatenate(
        [np.asarray(w_combine, np.float32).T, np.asarray(b_combine, np.float32)[None]],
        axis=0,
    ).astype(np.float32)
    wcb = np.zeros((P, E), np.float16)
    for st in range(4):
        wcb[32 * st : 32 * st + E9] = wcb9.astype(np.float16)
    sel = np.zeros((P, E9), np.float16)
    for st in range(4):
        for e in range(E9):
            sel[32 * st + e, e] = 1.0
    return thp, wcb, sel


_NC_CACHE = {}


def kernel(x, theta, w_combine, b_combine):
    from concourse.bass_utils import run_bass_kernel_spmd

    x = np.asarray(x, np.float32)
    B, S, _ = x.shape
    NCORES = 8
    NB = B // NCORES
    key = (S, NB)
    if key not in _NC_CACHE:
        _NC_CACHE[key] = build_nc(S=S, NB=NB)
    nc = _NC_CACHE[key]
    thp, wcb, sel = host_inputs(theta, w_combine, b_combine)
    in_maps = [
        {"x": x[c * NB : (c + 1) * NB], "thp": thp, "wcb": wcb, "sel": sel}
        for c in range(NCORES)
    ]
    res = run_bass_kernel_spmd(nc, in_maps, list(range(NCORES))).results
    return np.concatenate([res[c]["y"] for c in range(NCORES)], axis=0)
